# revision 1
# baseline (speedup 1.0000x reference)
"""Trainium2 Bass kernel for nn_Attention (LN -> QKV proj -> partial RoPE ->
null-KV prepend -> causal MQA attention -> out proj).

Sharding: 8 cores = 4 batches x 2 head-groups (8 heads each). Each core
computes its batch's LN/projections and its 8 heads' attention + partial
out-projection (through its W_out row-slice). Host sums the two head-group
partials per batch and stacks batches.

All compute ops keep uniform start-partitions (walrus checkSBSameStartPartition):
- k is projected twice (rows 0:64 and 64:128) so odd heads' QK matmuls run with
  lhsT/rhs both at base 64.
- rot projections are padded to pair layout so rope combines are base-aligned.
- the only cross-partition moves are SBUF->SBUF DMAs (odd-head attn-out rows,
  softmax-denominator row) and PE transposes.
"""

import sys

for _p in ("/opt/trn_rl_repo",):
    if _p not in sys.path:
        sys.path.insert(0, _p)

import numpy as np
import ml_dtypes

import concourse.bass as bass
import concourse.tile as tile
from concourse import bacc, mybir
from concourse import bass_utils

BF16 = ml_dtypes.bfloat16
F32 = np.float32

B, N, DIM = 4, 1024, 1024
HEADS, DH = 16, 64          # total heads; per-core 8
HPC = 8                     # heads per core
ROT = 32
NN = 2                      # null kv
EPS = 1e-5
P = 128
NEG = -1.0e38
SCALE = DH ** -0.5
NT = N // P                 # 8 i-tiles / D-chunks
IB = N // 512               # 2 i-blocks

dt = mybir.dt


def _chunks_for_block(b0):
    """j-tile chunks per i-block: lists of seq j-tile indices; 'T' = tail."""
    if b0 == 0:
        return [[0, 1], [2, 3], ["T"]]
    return [[0, 1], [2, 3], [4, 5], [6, 7], ["T"]]


def _prime_act_tables(arch):
    """Make Exp/Ln resolve to the single set containing both, so the
    act-table insertion pass emits one load instead of thrashing."""
    import concourse.hw_specs as hw_specs
    AF = mybir.ActivationFunctionType
    tables = hw_specs.get_activation_tables(arch)
    if "natural_log_exp_and_others" in tables:
        for name, fns in tables.items():
            if name != "natural_log_exp_and_others":
                fns.discard(AF.Exp)
                fns.discard(AF.Ln)


def _build_program(mask_trivial):
    nc = bacc.Bacc("TRN2", target_bir_lowering=False, debug=False)
    _prime_act_tables(nc.m.arch)

    f32, bf16 = dt.float32, dt.bfloat16
    AF = mybir.ActivationFunctionType
    OP = mybir.AluOpType

    d_x = nc.dram_tensor("x", [N, DIM], f32, kind="ExternalInput")
    d_wq = nc.dram_tensor("wq", [DIM, HPC * DH], bf16, kind="ExternalInput")
    # padded pair layout: per pair [even_rot(32), 0, odd_rot(32), 0]
    d_wqr = nc.dram_tensor("wqrot", [DIM, 4 * P], bf16, kind="ExternalInput")
    d_wkk = nc.dram_tensor("wkk", [DIM, P], bf16, kind="ExternalInput")   # [Wk|Wk]
    d_wv = nc.dram_tensor("wv", [DIM, DH], bf16, kind="ExternalInput")
    d_wkr = nc.dram_tensor("wkrot", [DIM, P], bf16, kind="ExternalInput")  # [krot,0,krot,0]
    d_wvr = nc.dram_tensor("wvrot", [DIM, DH], bf16, kind="ExternalInput")  # [vrot,0]
    d_wout = nc.dram_tensor("wout", [HPC * DH, DIM], bf16, kind="ExternalInput")
    d_cos = nc.dram_tensor("cosr", [P, N], bf16, kind="ExternalInput")
    d_sin = nc.dram_tensor("sinr", [P, N], bf16, kind="ExternalInput")
    d_tri = nc.dram_tensor("tri", [P, 5 * 512], bf16, kind="ExternalInput")
    d_ktail = nc.dram_tensor("ktail", [P, P], bf16, kind="ExternalInput")  # dup rows
    d_vtail = nc.dram_tensor("vtail", [P, DH + 2], bf16, kind="ExternalInput")
    d_qb = nc.dram_tensor("qbias", [P, 4], f32, kind="ExternalInput")
    d_qrb = nc.dram_tensor("qrotbias", [P, 4], f32, kind="ExternalInput")
    d_kb = nc.dram_tensor("kbias", [P, 1], f32, kind="ExternalInput")
    d_krb = nc.dram_tensor("krotbias", [P, 1], f32, kind="ExternalInput")
    d_vb = nc.dram_tensor("vbias", [DH, 1], f32, kind="ExternalInput")
    d_vrb = nc.dram_tensor("vrotbias", [DH, 1], f32, kind="ExternalInput")
    d_id = nc.dram_tensor("identm", [P, P], bf16, kind="ExternalInput")
    d_mb = None
    if not mask_trivial:
        d_mb = nc.dram_tensor("maskbias", [P, NT * 512], bf16, kind="ExternalInput")
    d_out = nc.dram_tensor("out", [N, DIM], f32, kind="ExternalOutput")

    with tile.TileContext(nc) as tc:
        from contextlib import ExitStack

        ctx = ExitStack()
        with ctx:
            consts = ctx.enter_context(tc.tile_pool(name="consts", bufs=1))
            persist = ctx.enter_context(tc.tile_pool(name="persist", bufs=1))

            # ---- persistent SBUF tensors ----
            wq_sb = consts.tile([P, NT * 512], bf16)       # 8 chunks x [128,512]
            wqr_sb = consts.tile([P, NT * 512], bf16)
            wkk_sb = consts.tile([P, NT * 128], bf16)
            wv_sb = consts.tile([P, NT * 64], bf16)
            wkr_sb = consts.tile([P, NT * 128], bf16)
            wvr_sb = consts.tile([P, NT * 64], bf16)
            wout_sb = consts.tile([P, 4 * DIM], bf16)      # 4 pair chunks
            cos_sb = consts.tile([P, N], bf16)
            sin_sb = consts.tile([P, N], bf16)
            tri_sb = consts.tile([P, 5 * 512], bf16)
            ktail_sb = consts.tile([P, P], bf16)
            vtail_sb = consts.tile([P, DH + 2], bf16)
            qb_sb = consts.tile([P, 4], f32)
            qrb_sb = consts.tile([P, 4], f32)
            kb_sb = consts.tile([P, 1], f32)
            krb_sb = consts.tile([P, 1], f32)
            vb_sb = consts.tile([DH, 1], f32)
            vrb_sb = consts.tile([DH, 1], f32)
            ident = consts.tile([P, P], bf16)
            mb_sb = None
            if not mask_trivial:
                mb_sb = consts.tile([P, NT * 512], bf16)

            xnT = persist.tile([P, NT * N], bf16)          # [D-chunk, i] chunks
            qp = persist.tile([P, 4 * N], bf16)            # q pairs [128, i]
            rotq = persist.tile([P, 4 * N], bf16)          # padded rot pairs
            kT = persist.tile([P, N], bf16)                # k duplicated rows
            rotk = persist.tile([P, N], bf16)
            vT = persist.tile([DH, N], bf16)
            rotv = persist.tile([DH, N], bf16)
            vext = persist.tile([P, 9 * (DH + 2)], bf16)   # v + dual ones cols
            ao = persist.tile([P, 4 * N], bf16)            # attn out pairs [128, i]
            nc.sync.dma_start(ident[:], d_id.ap()[:])

            # ======== Phases 1+2 interleaved: LN/transpose per i-half, then
            # ======== that half's projections+rope+vext, so PE overlaps LN.
            weights_loaded = [False]

            def load_weights():
                if weights_loaded[0]:
                    return
                weights_loaded[0] = True
                for c in range(NT):
                    sl = slice(c * P, (c + 1) * P)
                    nc.sync.dma_start(wq_sb[:, c * 512:(c + 1) * 512],
                                      d_wq.ap()[sl, :])
                    nc.sync.dma_start(wqr_sb[:, c * 512:(c + 1) * 512],
                                      d_wqr.ap()[sl, :])
                    nc.sync.dma_start(wkk_sb[:, c * 128:(c + 1) * 128],
                                      d_wkk.ap()[sl, :])
                    nc.sync.dma_start(wv_sb[:, c * 64:(c + 1) * 64],
                                      d_wv.ap()[sl, :])
                    nc.sync.dma_start(wkr_sb[:, c * 128:(c + 1) * 128],
                                      d_wkr.ap()[sl, :])
                    nc.sync.dma_start(wvr_sb[:, c * 64:(c + 1) * 64],
                                      d_wvr.ap()[sl, :])
                for p in range(4):
                    nc.sync.dma_start(wout_sb[:, p * DIM:(p + 1) * DIM],
                                      d_wout.ap()[p * P:(p + 1) * P, :])
                nc.sync.dma_start(cos_sb[:], d_cos.ap()[:])
                nc.sync.dma_start(sin_sb[:], d_sin.ap()[:])
                nc.sync.dma_start(tri_sb[:], d_tri.ap()[:])
                nc.sync.dma_start(ktail_sb[:], d_ktail.ap()[:])
                nc.sync.dma_start(vtail_sb[:], d_vtail.ap()[:])
                nc.sync.dma_start(qb_sb[:], d_qb.ap()[:])
                nc.sync.dma_start(qrb_sb[:], d_qrb.ap()[:])
                nc.sync.dma_start(kb_sb[:], d_kb.ap()[:])
                nc.sync.dma_start(krb_sb[:], d_krb.ap()[:])
                nc.sync.dma_start(vb_sb[:], d_vb.ap()[:])
                nc.sync.dma_start(vrb_sb[:], d_vrb.ap()[:])
                if not mask_trivial:
                    nc.sync.dma_start(mb_sb[:], d_mb.ap()[:])

            def ln_reduce_tile(ph1, stp, t, xt, rsums, accs):
                c4 = t % 4
                nc.vector.tensor_reduce(rsums[:, c4:c4 + 1], xt[:],
                                        axis=mybir.AxisListType.X, op=OP.add)
                sq = ph1.tile([P, DIM], bf16, tag="sq", name="sq")
                nc.scalar.activation(sq[:], xt[:], AF.Square,
                                     accum_out=accs[:, c4:c4 + 1])

            def ln_stats_batch(stp, rsums, accs):
                """Batched [128,4] stats for one 4-tile half -> (rstd, negmr)."""
                mean = stp.tile([P, 4], f32, tag="stb", name="mean")
                nc.vector.tensor_scalar(out=mean[:], in0=rsums[:],
                                        scalar1=1.0 / DIM, scalar2=None,
                                        op0=OP.mult)
                ex2 = stp.tile([P, 4], f32, tag="stb", name="ex2")
                nc.vector.tensor_scalar(out=ex2[:], in0=accs[:],
                                        scalar1=1.0 / DIM, scalar2=None,
                                        op0=OP.mult)
                var = stp.tile([P, 4], f32, tag="stb", name="var")
                nc.vector.scalar_tensor_tensor(
                    out=var[:], in0=mean[:], scalar=-1.0, in1=mean[:],
                    op0=OP.mult, op1=OP.mult)
                nc.vector.scalar_tensor_tensor(
                    out=var[:], in0=ex2[:], scalar=EPS, in1=var[:],
                    op0=OP.add, op1=OP.add)
                nc.scalar.activation(var[:], var[:], AF.Ln)
                rstd = stp.tile([P, 4], f32, tag="stb", name="rstd")
                nc.scalar.activation(rstd[:], var[:], AF.Exp, scale=-0.5)
                negmr = stp.tile([P, 4], f32, tag="stb", name="negmr")
                nc.vector.scalar_tensor_tensor(
                    out=negmr[:], in0=mean[:], scalar=-1.0, in1=rstd[:],
                    op0=OP.mult, op1=OP.mult)
                return rstd, negmr

            def ln_xn_tile(ph1, ps1, t, xt, rstd, negmr):
                c4 = t % 4
                xn = ph1.tile([P, DIM], bf16, tag="xn", name="xn")
                nc.vector.tensor_scalar(out=xn[:], in0=xt[:],
                                        scalar1=rstd[:, c4:c4 + 1],
                                        scalar2=negmr[:, c4:c4 + 1],
                                        op0=OP.mult, op1=OP.add)
                for g in range(2):
                    pst = ps1.tile([P, 512], bf16, tag="tp", name="pst")
                    for c4b in range(4):
                        c = g * 4 + c4b
                        nc.tensor.transpose(pst[:, c4b * P:(c4b + 1) * P],
                                            xn[:, c * P:(c + 1) * P], ident[:])
                    dest = xnT[:].rearrange("p (c i) -> p c i", c=NT)[
                        :, g * 4:(g + 1) * 4, t * P:(t + 1) * P]
                    src = pst[:].rearrange("p (c i) -> p c i", c=4)
                    nc.vector.tensor_copy(dest, src)

            def mm_proj(ps2, w_sb, wwidth, col0, cols, ib, rows=P):
                ps = ps2.tile([P, 512], f32, tag="proj", name="ps")
                for c in range(NT):
                    nc.tensor.matmul(
                        ps[0:rows, :],
                        w_sb[:, c * wwidth + col0: c * wwidth + col0 + cols],
                        xnT[:, c * N + ib * 512: c * N + ib * 512 + 512],
                        start=(c == 0), stop=(c == NT - 1))
                return ps

            def proj_ib(ps2, vtp, ib):
                isl = slice(ib * 512, (ib + 1) * 512)
                for p in range(4):
                    csl = slice(p * N + ib * 512, p * N + ib * 512 + 512)
                    ps = mm_proj(ps2, wq_sb, 512, p * P, P, ib)
                    nc.vector.tensor_scalar(
                        out=qp[:, csl], in0=ps[:], scalar1=qb_sb[:, p:p + 1],
                        scalar2=None, op0=OP.add)
                    ps = mm_proj(ps2, wqr_sb, 512, p * P, P, ib)
                    nc.vector.tensor_scalar(
                        out=rotq[:, csl], in0=ps[:], scalar1=qrb_sb[:, p:p + 1],
                        scalar2=None, op0=OP.add)
                ps = mm_proj(ps2, wkk_sb, 128, 0, P, ib)
                nc.vector.tensor_scalar(out=kT[:, isl], in0=ps[:],
                                        scalar1=kb_sb[:], scalar2=None,
                                        op0=OP.add)
                ps = mm_proj(ps2, wkr_sb, 128, 0, P, ib)
                nc.vector.tensor_scalar(out=rotk[:, isl], in0=ps[:],
                                        scalar1=krb_sb[:], scalar2=None,
                                        op0=OP.add)
                ps = mm_proj(ps2, wv_sb, 64, 0, DH, ib, rows=DH)
                nc.vector.tensor_scalar(out=vT[:, isl], in0=ps[0:DH, :],
                                        scalar1=vb_sb[:], scalar2=None,
                                        op0=OP.add)
                ps = mm_proj(ps2, wvr_sb, 64, 0, DH, ib, rows=DH)
                nc.vector.tensor_scalar(out=rotv[:, isl], in0=ps[0:DH, :],
                                        scalar1=vrb_sb[:], scalar2=None,
                                        op0=OP.add)
                # rope for this i-block
                nc.vector.tensor_tensor(out=rotk[:, isl], in0=rotk[:, isl],
                                        in1=sin_sb[:, isl], op=OP.mult)
                nc.vector.tensor_tensor(out=rotv[0:ROT, isl],
                                        in0=rotv[0:ROT, isl],
                                        in1=sin_sb[0:ROT, isl], op=OP.mult)
                for p in range(4):
                    csl = slice(p * N + ib * 512, p * N + ib * 512 + 512)
                    nc.vector.tensor_tensor(out=rotq[:, csl], in0=rotq[:, csl],
                                            in1=sin_sb[:, isl], op=OP.mult)
                for base in (0, DH):
                    rsl = slice(base, base + ROT)
                    for p in range(4):
                        csl = slice(p * N + ib * 512, p * N + ib * 512 + 512)
                        nc.vector.tensor_tensor(
                            out=qp[rsl, csl], in0=qp[rsl, csl],
                            in1=cos_sb[rsl, isl], op=OP.mult)
                        nc.vector.tensor_tensor(
                            out=qp[rsl, csl], in0=qp[rsl, csl],
                            in1=rotq[rsl, csl], op=OP.add)
                    nc.vector.tensor_tensor(out=kT[rsl, isl],
                                            in0=kT[rsl, isl],
                                            in1=cos_sb[rsl, isl], op=OP.mult)
                    nc.vector.tensor_tensor(out=kT[rsl, isl],
                                            in0=kT[rsl, isl],
                                            in1=rotk[rsl, isl], op=OP.add)
                nc.vector.tensor_tensor(out=vT[0:ROT, isl],
                                        in0=vT[0:ROT, isl],
                                        in1=cos_sb[0:ROT, isl], op=OP.mult)
                nc.vector.tensor_tensor(out=vT[0:ROT, isl],
                                        in0=vT[0:ROT, isl],
                                        in1=rotv[0:ROT, isl], op=OP.add)
                # v row-major + dual ones cols for this i-block's j-tiles
                for jj in range(ib * 4, ib * 4 + 4):
                    pv = vtp.tile([P, DH], bf16, tag="vt", name="pv")
                    nc.tensor.transpose(pv[:], vT[:, jj * P:(jj + 1) * P],
                                        ident[0:DH, 0:DH])
                    vbase = jj * (DH + 2)
                    nc.vector.tensor_copy(vext[:, vbase:vbase + DH], pv[:])
                    nc.vector.memset(vext[:, vbase + DH:vbase + DH + 2], 1.0)

            with tc.tile_pool(name="ph1sb", bufs=6) as ph1, \
                 tc.tile_pool(name="ph1st", bufs=32) as stp, \
                 tc.tile_pool(name="ph1ps", bufs=2, space="PSUM") as ps1, \
                 tc.tile_pool(name="ph2ps", bufs=5, space="PSUM") as ps2, \
                 tc.tile_pool(name="vtp", bufs=1, space="PSUM") as vtp:
                xts = []
                for t in range(NT):
                    xt = ph1.tile([P, DIM], f32, tag=f"x{t}", name=f"xt{t}",
                                  bufs=1)
                    nc.sync.dma_start(xt[:], d_x.ap()[t * P:(t + 1) * P, :])
                    xts.append(xt)
                load_weights()
                for half in range(2):
                    rsums = stp.tile([P, 4], f32, tag=f"rs{half}",
                                     name=f"rsums{half}", bufs=1)
                    accs = stp.tile([P, 4], f32, tag=f"ac{half}",
                                    name=f"accs{half}", bufs=1)
                    for t in range(half * 4, half * 4 + 4):
                        ln_reduce_tile(ph1, stp, t, xts[t], rsums, accs)
                    rstd, negmr = ln_stats_batch(stp, rsums, accs)
                    for t in range(half * 4, half * 4 + 4):
                        ln_xn_tile(ph1, ps1, t, xts[t], rstd, negmr)
                    proj_ib(ps2, vtp, half)
                nc.vector.tensor_copy(vext[:, 8 * (DH + 2):9 * (DH + 2)],
                                      vtail_sb[:])

            # ================= Phase 3: attention (pair-packed) =================
            # Even/odd heads of a pair run their K=64 QK matmuls on disjoint
            # PE row-groups (lhsT/rhs at base 0 vs 64) -> concurrent on the
            # systolic array. PSUM: 2 sim tags x 1 buf x 3 banks + 2 outT
            # tags x 1 buf x 1 bank = 8 banks.
            with tc.tile_pool(name="simps", bufs=3, space="PSUM") as simps, \
                 tc.tile_pool(name="outps", bufs=1, space="PSUM") as outps, \
                 tc.tile_pool(name="atsb", bufs=6) as atsb, \
                 tc.tile_pool(name="nrm", bufs=3) as nrm:
                for pc in range(4):
                    rsb = nrm.tile([P, N], f32, name="rsb", tag="rsb")
                    nc.vector.memset(rsb[DH:DH + ROT, :], 1.0)
                    aots = {}
                    for b0 in range(IB):
                        chunks = _chunks_for_block(b0)
                        alljj = [jj for ch in chunks for jj in ch]
                        qhs = {}
                        psos = {}
                        for e in (0, 1):
                            hb = e * DH
                            qhs[e] = qp[hb:hb + DH,
                                        pc * N + b0 * 512: pc * N + b0 * 512 + 512]
                            psos[e] = outps.tile([P, 512], f32,
                                                 name=f"pso{e}", tag=f"outT{e}")
                        first_av = True
                        for ch in chunks:
                            w = len(ch) * 512
                            pss = {}
                            for e in (0, 1):
                                pss[e] = simps.tile([P, 1024], f32,
                                                    name=f"pss{e}", tag="sim")
                            # interleaved sims: row-group concurrent per jj
                            for idx, jj in enumerate(ch):
                                for e in (0, 1):
                                    hb = e * DH
                                    seg = pss[e][:, idx * 512:(idx + 1) * 512]
                                    diag = jj != "T" and jj >= 4 * b0
                                    extra = (1 if jj == "T" else
                                             (1 if diag else 0)
                                             + (0 if mask_trivial else 1))
                                    if jj == "T":
                                        nc.tensor.matmul(
                                            seg, ktail_sb[hb:hb + DH, :], qhs[e],
                                            start=True, stop=False)
                                    else:
                                        nc.tensor.matmul(
                                            seg,
                                            kT[hb:hb + DH, jj * P:(jj + 1) * P],
                                            qhs[e], start=True, stop=(extra == 0))
                            # bias adds (K=128 identity matmuls)
                            for idx, jj in enumerate(ch):
                                for e in (0, 1):
                                    seg = pss[e][:, idx * 512:(idx + 1) * 512]
                                    if jj == "T":
                                        nc.tensor.matmul(
                                            seg, ident[:],
                                            tri_sb[:, 4 * 512:5 * 512],
                                            start=False, stop=True)
                                        continue
                                    diag = jj >= 4 * b0
                                    extra = ((1 if diag else 0)
                                             + (0 if mask_trivial else 1))
                                    if diag:
                                        k = jj - 4 * b0
                                        extra -= 1
                                        nc.tensor.matmul(
                                            seg, ident[:],
                                            tri_sb[:, k * 512:(k + 1) * 512],
                                            start=False, stop=(extra == 0))
                                    if not mask_trivial:
                                        extra -= 1
                                        nc.tensor.matmul(
                                            seg, ident[:],
                                            mb_sb[:, jj * 512:(jj + 1) * 512],
                                            start=False, stop=(extra == 0))
                            ats = {}
                            for e in (0, 1):
                                at = atsb.tile([P, 1024], bf16,
                                               name=f"at{e}", tag=f"at{e}")
                                nc.scalar.activation(at[:, 0:w], pss[e][:, 0:w],
                                                     AF.Exp, scale=SCALE)
                                ats[e] = at
                            for idx, jj in enumerate(ch):
                                vjj = 8 if jj == "T" else jj
                                vcols = vext[:, vjj * (DH + 2):(vjj + 1) * (DH + 2)]
                                for e in (0, 1):
                                    nc.tensor.matmul(
                                        psos[e][0:DH + 2, :], vcols,
                                        ats[e][:, idx * 512:(idx + 1) * 512],
                                        start=first_av,
                                        stop=(jj == alljj[-1]))
                                first_av = first_av and False if idx >= 0 else first_av
                            first_av = False
                        # evacuate psum promptly; sums row stays base-64
                        bsl0 = slice(b0 * 512, (b0 + 1) * 512)
                        for e in (1, 0):  # odd writes rows 64:66, even row 64
                            aot = nrm.tile([DH + 2, 512], f32,
                                           name=f"aot{b0}{e}", tag=f"aot{b0}{e}")
                            nc.vector.tensor_copy(aot[:], psos[e][0:DH + 2, :])
                            if e == 1:
                                nc.vector.tensor_copy(rsb[DH:DH + 2, bsl0],
                                                      aot[DH:DH + 2, :])
                            else:
                                nc.vector.tensor_copy(rsb[DH:DH + 1, bsl0],
                                                      aot[DH:DH + 1, :])
                            aots[(b0, e)] = aot
                    # one recip for both parities (rows 64/65), then bcast
                    rows2 = rsb[DH:DH + 2, :]
                    nc.scalar.activation(rows2, rows2, AF.Ln)
                    nc.scalar.activation(rows2, rows2, AF.Exp, scale=-1.0)
                    for e in (0, 1):
                        bc = nrm.tile([P, N], f32, name=f"bc{e}", tag=f"bc{e}")
                        nc.vector.stream_shuffle(bc[DH:DH + ROT, :],
                                                 rsb[DH:DH + ROT, :], [e] * 32)
                        nc.sync.dma_start(bc[0:ROT, :], bc[DH:DH + ROT, :])
                        nc.sync.dma_start(bc[ROT:DH, :], bc[0:ROT, :])
                        for b0 in range(IB):
                            osl = slice(pc * N + b0 * 512,
                                        pc * N + b0 * 512 + 512)
                            bsl = slice(b0 * 512, (b0 + 1) * 512)
                            src = aots[(b0, e)]
                            if e == 0:
                                nc.gpsimd.tensor_tensor(
                                    out=ao[0:DH, osl], in0=src[0:DH, :],
                                    in1=bc[0:DH, bsl], op=OP.mult)
                            else:
                                tmp = nrm.tile([DH, 512], bf16, name="tmpn",
                                               tag="tmpn")
                                nc.gpsimd.tensor_tensor(
                                    out=tmp[:], in0=src[0:DH, :],
                                    in1=bc[0:DH, bsl], op=OP.mult)
                                nc.sync.dma_start(ao[DH:P, osl], tmp[:])

            # ================= Phase 4: out projection =================
            with tc.tile_pool(name="opps", bufs=4, space="PSUM") as opps, \
                 tc.tile_pool(name="opsb", bufs=3) as opsb:
                for t in range(NT):
                    orow = opsb.tile([P, DIM], f32, tag="orow")
                    for nb in range(2):
                        ps = opps.tile([P, 512], f32, tag="op")
                        for p in range(4):
                            nc.tensor.matmul(
                                ps[:],
                                ao[:, p * N + t * P: p * N + t * P + 128],
                                wout_sb[:, p * DIM + nb * 512: p * DIM + nb * 512 + 512],
                                start=(p == 0), stop=(p == 3))
                        nc.vector.tensor_copy(orow[:, nb * 512:(nb + 1) * 512],
                                              ps[:])
                    nc.sync.dma_start(d_out.ap()[t * P:(t + 1) * P, :], orow[:])

    nc.compile()
    return nc


_PROG_CACHE = {}


def _get_program(mask_trivial):
    key = bool(mask_trivial)
    if key not in _PROG_CACHE:
        _PROG_CACHE[key] = _build_program(key)
    return _PROG_CACHE[key]


def _rot_cols(Wb):
    """rotate_half on the output-dim axis of a [..., ROT] block:
    rot(t)[0:16] = -t[16:32]; rot(t)[16:32] = t[0:16]."""
    half = ROT // 2
    out = np.empty_like(Wb)
    out[..., 0:half] = -Wb[..., half:ROT]
    out[..., half:ROT] = Wb[..., 0:half]
    return out


def _host_prep(core, x, mask, freqs, ln_g, ln_b, W_q, W_kv, W_out, null_kv,
               mask_trivial):
    b, g = core // 2, core % 2
    heads = slice(g * HPC * DH, (g + 1) * HPC * DH)

    Wq_eff = (W_q * ln_g[:, None])[:, heads]            # [1024, 512]
    Wkv_eff = W_kv * ln_g[:, None]                      # [1024, 128]
    Wout_g = W_out[heads, :]                            # [512, 1024]
    bq = (ln_b @ W_q)[heads]                            # [512]
    bkv = ln_b @ W_kv                                   # [128]
    Wk, Wv = Wkv_eff[:, 0:DH], Wkv_eff[:, DH:2 * DH]
    bk, bv = bkv[0:DH], bkv[DH:2 * DH]

    # padded-pair rot weights for q: per pair [even_rot, 0, odd_rot, 0]
    wqrot = np.zeros((DIM, 4 * P), np.float64)
    qrotbias = np.zeros((P, 4), F32)
    for h in range(HPC):
        p, e = divmod(h, 2)
        blk = Wq_eff[:, h * DH: h * DH + ROT]
        wqrot[:, p * P + e * DH: p * P + e * DH + ROT] = _rot_cols(blk)
        qrotbias[e * DH:e * DH + ROT, p] = _rot_cols(bq[h * DH: h * DH + ROT])
    # duplicated k / krot (rows 0:64 == 64:128)
    wkk = np.concatenate([Wk, Wk], 1)
    kbias = np.concatenate([bk, bk]).reshape(P, 1).astype(F32)
    wkrot = np.zeros((DIM, P), np.float64)
    wkrot[:, 0:ROT] = _rot_cols(Wk[:, 0:ROT])
    wkrot[:, DH:DH + ROT] = wkrot[:, 0:ROT]
    krotbias = np.zeros((P, 1), F32)
    krotbias[0:ROT, 0] = _rot_cols(bk[0:ROT])
    krotbias[DH:DH + ROT, 0] = krotbias[0:ROT, 0]
    wvrot = np.zeros((DIM, DH), np.float64)
    wvrot[:, 0:ROT] = _rot_cols(Wv[:, 0:ROT])
    vrotbias = np.zeros((DH, 1), F32)
    vrotbias[0:ROT, 0] = _rot_cols(bv[0:ROT])

    qbias = np.zeros((P, 4), F32)
    for p in range(4):
        qbias[:, p] = bq[p * 128:(p + 1) * 128]

    f = np.asarray(freqs, np.float64)                   # [1024, 32]
    cosr = np.tile(np.cos(f).T, (4, 1))                 # [128, 1024]
    sinr = np.tile(np.sin(f).T, (4, 1))

    tri = np.zeros((P, 5 * 512), F32)
    pidx = np.arange(P)[:, None]
    il = np.arange(512)[None, :]
    for k in range(4):
        tri[:, k * 512:(k + 1) * 512] = np.where(il >= 128 * k + pidx, 0.0, NEG)
    tri[NN:, 4 * 512:5 * 512] = NEG                     # tail: rows >= 2 masked

    ktail = np.zeros((P, P), F32)
    nk = np.asarray(null_kv[0]).T                       # [64, 2]
    ktail[0:DH, 0:NN] = nk
    ktail[DH:P, 0:NN] = nk
    vtail = np.zeros((P, DH + 2), F32)
    vtail[0:NN, 0:DH] = np.asarray(null_kv[1])
    vtail[0:NN, DH:DH + 2] = 1.0

    im = {
        "x": np.ascontiguousarray(x[b], F32),
        "wq": Wq_eff.astype(BF16),
        "wqrot": wqrot.astype(BF16),
        "wkk": wkk.astype(BF16),
        "wv": np.ascontiguousarray(Wv).astype(BF16),
        "wkrot": wkrot.astype(BF16),
        "wvrot": wvrot.astype(BF16),
        "wout": np.ascontiguousarray(Wout_g).astype(BF16),
        "cosr": cosr.astype(BF16),
        "sinr": sinr.astype(BF16),
        "tri": tri.astype(BF16),
        "ktail": ktail.astype(BF16),
        "vtail": vtail.astype(BF16),
        "qbias": qbias,
        "qrotbias": qrotbias,
        "kbias": kbias,
        "krotbias": krotbias,
        "vbias": bv.reshape(DH, 1).astype(F32),
        "vrotbias": vrotbias,
        "identm": np.eye(P, dtype=BF16),
    }
    if not mask_trivial:
        mrow = np.where(np.asarray(mask[b]), 0.0, NEG)  # [1024]
        mb = np.zeros((P, NT * 512), F32)
        for jj in range(NT):
            mb[:, jj * 512:(jj + 1) * 512] = mrow[jj * P:(jj + 1) * P][:, None]
        im["maskbias"] = mb.astype(BF16)
    return im


def _run(x, mask, freqs, ln_g, ln_b, W_q, W_kv, W_out, null_kv, **spmd_kwargs):
    x = np.asarray(x, F32)
    mask = np.asarray(mask)
    freqs = np.asarray(freqs, F32)
    ln_g = np.asarray(ln_g, np.float64)
    ln_b = np.asarray(ln_b, np.float64)
    W_q = np.asarray(W_q, np.float64)
    W_kv = np.asarray(W_kv, np.float64)
    W_out = np.asarray(W_out, np.float64)
    null_kv = np.asarray(null_kv, F32)

    mask_trivial = bool(mask.all())
    nc = _get_program(mask_trivial)
    in_maps = [
        _host_prep(c, x, mask, freqs, ln_g, ln_b, W_q, W_kv, W_out, null_kv,
                   mask_trivial)
        for c in range(8)
    ]
    res = bass_utils.run_bass_kernel_spmd(nc, in_maps, list(range(8)),
                                          **spmd_kwargs)
    out = np.empty((B, N, DIM), F32)
    for b in range(B):
        out[b] = res.results[2 * b]["out"] + res.results[2 * b + 1]["out"]
    return out, res


def kernel(x, mask, freqs, ln_g, ln_b, W_q, W_kv, W_out, null_kv):
    out, _ = _run(x, mask, freqs, ln_g, ln_b, W_q, W_kv, W_out, null_kv)
    return out



# revision 11
# speedup vs baseline: 3.7656x; 3.7656x over previous
"""Trainium2 Bass kernel for nn_Attention (LN -> QKV proj -> partial RoPE ->
null-KV prepend -> causal MQA attention -> out proj).

Dispatch-cost-aware sharding: the axon PJRT path costs ~10ms fixed +
~0.9ms/core + ~0.8ms/buffer per exec, with payload bytes nearly free.
So: NCORES cores (default 2), each computing NB=4//NCORES full batches
(all 16 heads), with ONE packed f32 input blob + ONE f32 output tensor
per core. Output is a disjoint batch stack (no host reduction).

RoPE is applied post-projection via stream_shuffle partition rotation of
the biased q/k/v rows (rot contribution = shuffle * signed-sin + q * cos),
so no separate rot-weight projections are needed.

All compute ops keep uniform start-partitions (walrus checkSBSameStartPartition):
- k is projected twice (rows 0:64 and 64:128) so odd heads' QK matmuls run with
  lhsT/rhs both at base 64.
- rope groups live at rows base+(0:32) for base in {0, 64}; shuffles and
  combines stay within one base.
"""

import sys

for _p in ("/opt/trn_rl_repo",):
    if _p not in sys.path:
        sys.path.insert(0, _p)

import numpy as np

import concourse.bass as bass
import concourse.tile as tile
from concourse import bacc, mybir
from concourse import bass_utils

F32 = np.float32

B, N, DIM = 4, 1024, 1024
HEADS, DH = 16, 64
PC = HEADS // 2             # 8 head-pair groups, all on one core
ROT = 32
NN = 2                      # null kv
EPS = 1e-5
P = 128
NEG = -1.0e38
SCALE = DH ** -0.5
NT = N // P                 # 8 i-tiles / D-chunks
IB = N // 512               # 2 i-blocks

NB = 2                      # batches per core
NCORES = B // NB

dt = mybir.dt

ROT_SHUF = list(range(16, 32)) + list(range(0, 16))


def _chunks_for_block(b0):
    """j-tile chunks per i-block: lists of seq j-tile indices; 'T' = tail."""
    if b0 == 0:
        return [[0, 1], [2, 3], ["T"]]
    return [[0, 1], [2, 3], [4, 5], [6, 7], ["T"]]


def _prime_act_tables(arch):
    """Make Exp/Ln resolve to the single set containing both, so the
    act-table insertion pass emits one load instead of thrashing."""
    import concourse.hw_specs as hw_specs
    AF = mybir.ActivationFunctionType
    tables = hw_specs.get_activation_tables(arch)
    if "natural_log_exp_and_others" in tables:
        for name, fns in tables.items():
            if name != "natural_log_exp_and_others":
                fns.discard(AF.Exp)
                fns.discard(AF.Ln)


def _blob_rows(nb, mask_trivial):
    """Row offsets of each section in the packed [R, 1024] f32 blob."""
    off = {}
    r = 0
    off["x"] = r; r += nb * N
    off["wq"] = r; r += DIM          # [1024, 1024]
    off["wkv"] = r; r += DIM         # cols 0:128 = [Wk|Wk], 128:192 = Wv
    off["wout"] = r; r += DIM        # [1024, 1024]
    off["cos"] = r; r += P           # [128, 1024]
    off["sinm"] = r; r += P          # signed sin, [128, 1024]
    off["tri"] = r; r += 3 * P       # [128,2560] as 3 bands (1024,1024,512)
    off["misc"] = r; r += P          # ktail|ident|vtail|qb|kb|vb
    if not mask_trivial:
        off["mb"] = r; r += nb * 4 * P   # per-batch [128, 4096] as 4 bands
    off["_total"] = r
    return off


# misc band column layout
MC_KTAIL = 0          # [128, 128]
MC_IDENT = 128        # [128, 128]
MC_VTAIL = 256        # [128, 66]
MC_QB = 322           # [128, 8] f32
MC_KB = 330           # [128, 1] f32
MC_VB = 331           # [64, 1] f32


def _build_program(nb, mask_trivial):
    nc = bacc.Bacc("TRN2", target_bir_lowering=False, debug=False)
    _prime_act_tables(nc.m.arch)

    f32, bf16 = dt.float32, dt.bfloat16
    AF = mybir.ActivationFunctionType
    OP = mybir.AluOpType

    R = _blob_rows(nb, mask_trivial)
    d_blob = nc.dram_tensor("blob", [R["_total"], 1024], f32,
                            kind="ExternalInput")
    d_out = nc.dram_tensor("out", [nb * N, DIM], f32, kind="ExternalOutput")

    def bap(key, r0, r1, c0, c1):
        return d_blob.ap()[R[key] + r0: R[key] + r1, c0:c1]

    with tile.TileContext(nc) as tc:
        from contextlib import ExitStack

        ctx = ExitStack()
        with ctx:
            consts = ctx.enter_context(tc.tile_pool(name="consts", bufs=1))
            persist = ctx.enter_context(tc.tile_pool(name="persist", bufs=1))

            # ---- persistent SBUF tensors ----
            wq_sb = consts.tile([P, NT * 1024], bf16)      # 8 chunks x [128,1024]
            wkk_sb = consts.tile([P, NT * 128], bf16)
            wv_sb = consts.tile([P, NT * 64], bf16)
            wout_sb = consts.tile([P, PC * DIM], bf16)     # 8 pair chunks
            cos_sb = consts.tile([P, N], bf16)
            sinm_sb = consts.tile([P, N], bf16)
            tri_sb = consts.tile([P, 5 * 512], bf16)
            ktail_sb = consts.tile([P, P], bf16)
            vtail_sb = consts.tile([P, DH + 2], bf16)
            ident = consts.tile([P, P], bf16)
            qb_sb = consts.tile([P, PC], f32)
            kb_sb = consts.tile([P, 1], f32)
            vb_sb = consts.tile([DH, 1], f32)
            mb_sb = None
            if not mask_trivial:
                mb_sb = persist.tile([P, NT * 512], bf16)

            qp = persist.tile([P, PC * N], bf16)           # q pairs [128, i]
            kT = persist.tile([P, N], bf16)                # k duplicated rows
            vT = persist.tile([DH, N], bf16)
            vext = persist.tile([P, 9 * (DH + 2)], bf16)   # v + dual ones cols
            ao = persist.tile([P, PC * N], bf16)           # attn out pairs

            # ---- load + convert weights (one blob -> bf16 SBUF consts) ----
            with tc.tile_pool(name="wstg", bufs=3) as stg:
                def load_conv(dst, r0, w, tag="stg"):
                    st = stg.tile([P, DIM], f32, tag=tag, name=tag)
                    nc.sync.dma_start(st[:, 0:w],
                                      d_blob.ap()[r0:r0 + P, 0:w])
                    nc.vector.tensor_copy(dst, st[:, 0:w])

                for c in range(NT):
                    load_conv(wq_sb[:, c * 1024:(c + 1) * 1024],
                              R["wq"] + c * P, 1024)
                for c in range(NT):
                    st = stg.tile([P, DIM], f32, tag="stg", name="stg")
                    nc.sync.dma_start(st[:, 0:192],
                                      bap("wkv", c * P, (c + 1) * P, 0, 192))
                    nc.vector.tensor_copy(wkk_sb[:, c * 128:(c + 1) * 128],
                                          st[:, 0:128])
                    nc.vector.tensor_copy(wv_sb[:, c * 64:(c + 1) * 64],
                                          st[:, 128:192])
                for p in range(PC):
                    load_conv(wout_sb[:, p * DIM:(p + 1) * DIM],
                              R["wout"] + p * P, 1024)
                load_conv(cos_sb[:], R["cos"], 1024)
                load_conv(sinm_sb[:], R["sinm"], 1024)
                load_conv(tri_sb[:, 0:1024], R["tri"], 1024)
                load_conv(tri_sb[:, 1024:2048], R["tri"] + P, 1024)
                load_conv(tri_sb[:, 2048:2560], R["tri"] + 2 * P, 512)
                st = stg.tile([P, DIM], f32, tag="stg", name="stg")
                nc.sync.dma_start(st[:, 0:MC_VTAIL + DH + 2],
                                  bap("misc", 0, P, 0, MC_VTAIL + DH + 2))
                nc.vector.tensor_copy(ktail_sb[:], st[:, MC_KTAIL:MC_KTAIL + P])
                nc.vector.tensor_copy(ident[:], st[:, MC_IDENT:MC_IDENT + P])
                nc.vector.tensor_copy(vtail_sb[:],
                                      st[:, MC_VTAIL:MC_VTAIL + DH + 2])
                nc.sync.dma_start(qb_sb[:], bap("misc", 0, P, MC_QB, MC_QB + PC))
                nc.sync.dma_start(kb_sb[:], bap("misc", 0, P, MC_KB, MC_KB + 1))
                nc.sync.dma_start(vb_sb[:], bap("misc", 0, DH, MC_VB, MC_VB + 1))

            # ---- helpers (same structure as 8-head version, PC=8) ----
            def ln_reduce_tile(ph1, t, xt, rsums, accs):
                c4 = t % 4
                nc.vector.tensor_reduce(rsums[:, c4:c4 + 1], xt[:],
                                        axis=mybir.AxisListType.X, op=OP.add)
                sq = ph1.tile([P, DIM], bf16, tag="sq", name="sq")
                nc.scalar.activation(sq[:], xt[:], AF.Square,
                                     accum_out=accs[:, c4:c4 + 1])

            def ln_stats_batch(stp, rsums, accs):
                mean = stp.tile([P, 4], f32, tag="stb", name="mean")
                nc.vector.tensor_scalar(out=mean[:], in0=rsums[:],
                                        scalar1=1.0 / DIM, scalar2=None,
                                        op0=OP.mult)
                ex2 = stp.tile([P, 4], f32, tag="stb", name="ex2")
                nc.vector.tensor_scalar(out=ex2[:], in0=accs[:],
                                        scalar1=1.0 / DIM, scalar2=None,
                                        op0=OP.mult)
                var = stp.tile([P, 4], f32, tag="stb", name="var")
                nc.vector.scalar_tensor_tensor(
                    out=var[:], in0=mean[:], scalar=-1.0, in1=mean[:],
                    op0=OP.mult, op1=OP.mult)
                nc.vector.scalar_tensor_tensor(
                    out=var[:], in0=ex2[:], scalar=EPS, in1=var[:],
                    op0=OP.add, op1=OP.add)
                nc.scalar.activation(var[:], var[:], AF.Ln)
                rstd = stp.tile([P, 4], f32, tag="stb", name="rstd")
                nc.scalar.activation(rstd[:], var[:], AF.Exp, scale=-0.5)
                negmr = stp.tile([P, 4], f32, tag="stb", name="negmr")
                nc.vector.scalar_tensor_tensor(
                    out=negmr[:], in0=mean[:], scalar=-1.0, in1=rstd[:],
                    op0=OP.mult, op1=OP.mult)
                return rstd, negmr

            def ln_xn_tile(xnT, ph1, ps1, t, xt, rstd, negmr):
                c4 = t % 4
                xn = ph1.tile([P, DIM], bf16, tag="xn", name="xn")
                nc.vector.tensor_scalar(out=xn[:], in0=xt[:],
                                        scalar1=rstd[:, c4:c4 + 1],
                                        scalar2=negmr[:, c4:c4 + 1],
                                        op0=OP.mult, op1=OP.add)
                for g in range(2):
                    pst = ps1.tile([P, 512], bf16, tag="tp", name="pst")
                    for c4b in range(4):
                        c = g * 4 + c4b
                        nc.tensor.transpose(pst[:, c4b * P:(c4b + 1) * P],
                                            xn[:, c * P:(c + 1) * P], ident[:])
                    dest = xnT[:].rearrange("p (c i) -> p c i", c=NT)[
                        :, g * 4:(g + 1) * 4, t * P:(t + 1) * P]
                    src = pst[:].rearrange("p (c i) -> p c i", c=4)
                    nc.vector.tensor_copy(dest, src)

            def mm_proj(xnT, ps2, w_sb, wwidth, col0, cols, ib, rows=P):
                ps = ps2.tile([P, 512], f32, tag="proj", name="ps")
                for c in range(NT):
                    nc.tensor.matmul(
                        ps[0:rows, :],
                        w_sb[:, c * wwidth + col0: c * wwidth + col0 + cols],
                        xnT[:, c * N + ib * 512: c * N + ib * 512 + 512],
                        start=(c == 0), stop=(c == NT - 1))
                return ps

            def rope_rows(rp, dst, base, isl_c, sin_cols):
                """dst rows base:base+32 (cols isl_c slice of width 512):
                dst = dst*cos + shuffle(dst)*sinm."""
                rsl = slice(base, base + ROT)
                tmp = rp.tile([P, 512], bf16, tag="rt", name="rt")
                nc.vector.stream_shuffle(tmp[rsl, :], dst[rsl, isl_c], ROT_SHUF)
                nc.vector.tensor_tensor(out=dst[rsl, isl_c], in0=dst[rsl, isl_c],
                                        in1=cos_sb[rsl, sin_cols], op=OP.mult)
                nc.vector.tensor_tensor(out=tmp[rsl, :], in0=tmp[rsl, :],
                                        in1=sinm_sb[rsl, sin_cols], op=OP.mult)
                nc.vector.tensor_tensor(out=dst[rsl, isl_c], in0=dst[rsl, isl_c],
                                        in1=tmp[rsl, :], op=OP.add)

            def proj_ib(xnT, ps2, vtp, rp, ib):
                isl = slice(ib * 512, (ib + 1) * 512)
                for p in range(PC):
                    csl = slice(p * N + ib * 512, p * N + ib * 512 + 512)
                    ps = mm_proj(xnT, ps2, wq_sb, 1024, p * P, P, ib)
                    nc.vector.tensor_scalar(
                        out=qp[:, csl], in0=ps[:], scalar1=qb_sb[:, p:p + 1],
                        scalar2=None, op0=OP.add)
                    for base in (0, DH):
                        rope_rows(rp, qp, base, csl, isl)
                ps = mm_proj(xnT, ps2, wkk_sb, 128, 0, P, ib)
                nc.vector.tensor_scalar(out=kT[:, isl], in0=ps[:],
                                        scalar1=kb_sb[:], scalar2=None,
                                        op0=OP.add)
                for base in (0, DH):
                    rope_rows(rp, kT, base, isl, isl)
                ps = mm_proj(xnT, ps2, wv_sb, 64, 0, DH, ib, rows=DH)
                nc.vector.tensor_scalar(out=vT[:, isl], in0=ps[0:DH, :],
                                        scalar1=vb_sb[:], scalar2=None,
                                        op0=OP.add)
                rope_rows(rp, vT, 0, isl, isl)
                # v row-major + dual ones cols for this i-block's j-tiles
                for jj in range(ib * 4, ib * 4 + 4):
                    pv = vtp.tile([P, DH], bf16, tag="vt", name="pv")
                    nc.tensor.transpose(pv[:], vT[:, jj * P:(jj + 1) * P],
                                        ident[0:DH, 0:DH])
                    vbase = jj * (DH + 2)
                    nc.vector.tensor_copy(vext[:, vbase:vbase + DH], pv[:])
                    nc.vector.memset(vext[:, vbase + DH:vbase + DH + 2], 1.0)

            # ================= per-batch pipeline =================
            for b in range(nb):
                if not mask_trivial:
                    with tc.tile_pool(name="mstg", bufs=2) as mstg:
                        for band in range(4):
                            st = mstg.tile([P, DIM], f32, tag="ms", name="ms")
                            nc.sync.dma_start(
                                st[:], bap("mb", (b * 4 + band) * P,
                                           (b * 4 + band + 1) * P, 0, 1024))
                            nc.vector.tensor_copy(
                                mb_sb[:, band * 1024:(band + 1) * 1024], st[:])

                # ---- Phases 1+2: LN + projections + rope ----
                with tc.tile_pool(name="ph1sb", bufs=4) as ph1, \
                     tc.tile_pool(name="ph1st", bufs=32) as stp, \
                     tc.tile_pool(name="xnp", bufs=1) as xnp, \
                     tc.tile_pool(name="ph1ps", bufs=2, space="PSUM") as ps1, \
                     tc.tile_pool(name="ph2ps", bufs=5, space="PSUM") as ps2, \
                     tc.tile_pool(name="rope", bufs=4) as rp, \
                     tc.tile_pool(name="vtp", bufs=1, space="PSUM") as vtp:
                    xnT = xnp.tile([P, NT * N], bf16, tag="xnT", name="xnT")
                    xts = []
                    for t in range(NT):
                        xt = ph1.tile([P, DIM], f32, tag=f"x{t % 4}",
                                      name=f"xt{t}", bufs=2)
                        nc.sync.dma_start(
                            xt[:], bap("x", b * N + t * P, b * N + (t + 1) * P,
                                       0, 1024))
                        xts.append(xt)
                    for half in range(2):
                        rsums = stp.tile([P, 4], f32, tag=f"rs{half}",
                                         name=f"rsums{half}", bufs=1)
                        accs = stp.tile([P, 4], f32, tag=f"ac{half}",
                                        name=f"accs{half}", bufs=1)
                        for t in range(half * 4, half * 4 + 4):
                            ln_reduce_tile(ph1, t, xts[t], rsums, accs)
                        rstd, negmr = ln_stats_batch(stp, rsums, accs)
                        for t in range(half * 4, half * 4 + 4):
                            ln_xn_tile(xnT, ph1, ps1, t, xts[t], rstd, negmr)
                        proj_ib(xnT, ps2, vtp, rp, half)
                    nc.vector.tensor_copy(vext[:, 8 * (DH + 2):9 * (DH + 2)],
                                          vtail_sb[:])

                # ---- Phase 3: attention (pair-packed) ----
                with tc.tile_pool(name="simps", bufs=3, space="PSUM") as simps, \
                     tc.tile_pool(name="outps", bufs=1, space="PSUM") as outps, \
                     tc.tile_pool(name="atsb", bufs=6) as atsb, \
                     tc.tile_pool(name="nrm", bufs=3) as nrm:
                    for pc in range(PC):
                        rsb = nrm.tile([P, N], f32, name="rsb", tag="rsb")
                        nc.vector.memset(rsb[DH:DH + ROT, :], 1.0)
                        aots = {}
                        for b0 in range(IB):
                            chunks = _chunks_for_block(b0)
                            alljj = [jj for ch in chunks for jj in ch]
                            qhs = {}
                            psos = {}
                            for e in (0, 1):
                                hb = e * DH
                                qhs[e] = qp[hb:hb + DH,
                                            pc * N + b0 * 512:
                                            pc * N + b0 * 512 + 512]
                                psos[e] = outps.tile([P, 512], f32,
                                                     name=f"pso{e}",
                                                     tag=f"outT{e}")
                            first_av = True
                            for ch in chunks:
                                w = len(ch) * 512
                                pss = {}
                                for e in (0, 1):
                                    pss[e] = simps.tile([P, 1024], f32,
                                                        name=f"pss{e}",
                                                        tag="sim")
                                for idx, jj in enumerate(ch):
                                    for e in (0, 1):
                                        hb = e * DH
                                        seg = pss[e][:, idx * 512:(idx + 1) * 512]
                                        diag = jj != "T" and jj >= 4 * b0
                                        extra = (1 if jj == "T" else
                                                 (1 if diag else 0)
                                                 + (0 if mask_trivial else 1))
                                        if jj == "T":
                                            nc.tensor.matmul(
                                                seg, ktail_sb[hb:hb + DH, :],
                                                qhs[e], start=True, stop=False)
                                        else:
                                            nc.tensor.matmul(
                                                seg,
                                                kT[hb:hb + DH,
                                                   jj * P:(jj + 1) * P],
                                                qhs[e], start=True,
                                                stop=(extra == 0))
                                for idx, jj in enumerate(ch):
                                    for e in (0, 1):
                                        seg = pss[e][:, idx * 512:(idx + 1) * 512]
                                        if jj == "T":
                                            nc.tensor.matmul(
                                                seg, ident[:],
                                                tri_sb[:, 4 * 512:5 * 512],
                                                start=False, stop=True)
                                            continue
                                        diag = jj >= 4 * b0
                                        extra = ((1 if diag else 0)
                                                 + (0 if mask_trivial else 1))
                                        if diag:
                                            k = jj - 4 * b0
                                            extra -= 1
                                            nc.tensor.matmul(
                                                seg, ident[:],
                                                tri_sb[:, k * 512:(k + 1) * 512],
                                                start=False, stop=(extra == 0))
                                        if not mask_trivial:
                                            extra -= 1
                                            nc.tensor.matmul(
                                                seg, ident[:],
                                                mb_sb[:, jj * 512:(jj + 1) * 512],
                                                start=False, stop=(extra == 0))
                                ats = {}
                                for e in (0, 1):
                                    at = atsb.tile([P, 1024], bf16,
                                                   name=f"at{e}", tag=f"at{e}")
                                    nc.scalar.activation(at[:, 0:w],
                                                         pss[e][:, 0:w],
                                                         AF.Exp, scale=SCALE)
                                    ats[e] = at
                                for idx, jj in enumerate(ch):
                                    vjj = 8 if jj == "T" else jj
                                    vcols = vext[:, vjj * (DH + 2):
                                                 (vjj + 1) * (DH + 2)]
                                    for e in (0, 1):
                                        nc.tensor.matmul(
                                            psos[e][0:DH + 2, :], vcols,
                                            ats[e][:, idx * 512:(idx + 1) * 512],
                                            start=first_av,
                                            stop=(jj == alljj[-1]))
                                    first_av = False
                            bsl0 = slice(b0 * 512, (b0 + 1) * 512)
                            for e in (1, 0):
                                aot = nrm.tile([DH + 2, 512], f32,
                                               name=f"aot{b0}{e}",
                                               tag=f"aot{b0}{e}")
                                nc.vector.tensor_copy(aot[:],
                                                      psos[e][0:DH + 2, :])
                                if e == 1:
                                    nc.vector.tensor_copy(rsb[DH:DH + 2, bsl0],
                                                          aot[DH:DH + 2, :])
                                else:
                                    nc.vector.tensor_copy(rsb[DH:DH + 1, bsl0],
                                                          aot[DH:DH + 1, :])
                                aots[(b0, e)] = aot
                        rows2 = rsb[DH:DH + 2, :]
                        nc.scalar.activation(rows2, rows2, AF.Ln)
                        nc.scalar.activation(rows2, rows2, AF.Exp, scale=-1.0)
                        for e in (0, 1):
                            bc = nrm.tile([P, N], f32, name=f"bc{e}",
                                          tag=f"bc{e}")
                            nc.vector.stream_shuffle(bc[DH:DH + ROT, :],
                                                     rsb[DH:DH + ROT, :],
                                                     [e] * 32)
                            nc.sync.dma_start(bc[0:ROT, :], bc[DH:DH + ROT, :])
                            nc.sync.dma_start(bc[ROT:DH, :], bc[0:ROT, :])
                            for b0 in range(IB):
                                osl = slice(pc * N + b0 * 512,
                                            pc * N + b0 * 512 + 512)
                                bsl = slice(b0 * 512, (b0 + 1) * 512)
                                src = aots[(b0, e)]
                                if e == 0:
                                    nc.gpsimd.tensor_tensor(
                                        out=ao[0:DH, osl], in0=src[0:DH, :],
                                        in1=bc[0:DH, bsl], op=OP.mult)
                                else:
                                    tmp = nrm.tile([DH, 512], bf16,
                                                   name="tmpn", tag="tmpn")
                                    nc.gpsimd.tensor_tensor(
                                        out=tmp[:], in0=src[0:DH, :],
                                        in1=bc[0:DH, bsl], op=OP.mult)
                                    nc.sync.dma_start(ao[DH:P, osl], tmp[:])

                # ---- Phase 4: out projection ----
                with tc.tile_pool(name="opps", bufs=4, space="PSUM") as opps, \
                     tc.tile_pool(name="opsb", bufs=3) as opsb:
                    for t in range(NT):
                        orow = opsb.tile([P, DIM], f32, tag="orow")
                        for nb2 in range(2):
                            ps = opps.tile([P, 512], f32, tag="op")
                            for p in range(PC):
                                nc.tensor.matmul(
                                    ps[:],
                                    ao[:, p * N + t * P: p * N + t * P + 128],
                                    wout_sb[:, p * DIM + nb2 * 512:
                                            p * DIM + nb2 * 512 + 512],
                                    start=(p == 0), stop=(p == PC - 1))
                            nc.vector.tensor_copy(
                                orow[:, nb2 * 512:(nb2 + 1) * 512], ps[:])
                        nc.sync.dma_start(
                            d_out.ap()[b * N + t * P: b * N + (t + 1) * P, :],
                            orow[:])

    nc.compile()
    return nc


_PROG_CACHE = {}


def _get_program(mask_trivial, nb=NB):
    key = (nb, bool(mask_trivial))
    if key not in _PROG_CACHE:
        _PROG_CACHE[key] = _build_program(nb, key[1])
    return _PROG_CACHE[key]


def _host_prep(core, x, mask, freqs, ln_g, ln_b, W_q, W_kv, W_out, null_kv,
               mask_trivial, nb=NB):
    R = _blob_rows(nb, mask_trivial)
    blob = np.zeros((R["_total"], 1024), F32)

    for i in range(nb):
        blob[R["x"] + i * N: R["x"] + (i + 1) * N, :] = x[core * nb + i]

    Wq_eff = W_q * ln_g[:, None]                        # [1024, 1024]
    Wkv_eff = W_kv * ln_g[:, None]                      # [1024, 128]
    bq = ln_b @ W_q                                     # [1024]
    bkv = ln_b @ W_kv                                   # [128]
    Wk, Wv = Wkv_eff[:, 0:DH], Wkv_eff[:, DH:2 * DH]
    bk, bv = bkv[0:DH], bkv[DH:2 * DH]

    blob[R["wq"]:R["wq"] + DIM, :] = Wq_eff
    blob[R["wkv"]:R["wkv"] + DIM, 0:DH] = Wk
    blob[R["wkv"]:R["wkv"] + DIM, DH:2 * DH] = Wk
    blob[R["wkv"]:R["wkv"] + DIM, 128:192] = Wv
    blob[R["wout"]:R["wout"] + DIM, :] = W_out

    f = np.asarray(freqs, np.float64)                   # [1024, 32]
    blob[R["cos"]:R["cos"] + P, :] = np.tile(np.cos(f).T, (4, 1))
    s = np.sin(f).T                                     # [32, 1024]
    sm = s.copy()
    sm[0:ROT // 2, :] = -s[0:ROT // 2, :]
    blob[R["sinm"]:R["sinm"] + P, :] = np.tile(sm, (4, 1))

    tri = np.zeros((P, 5 * 512), F32)
    pidx = np.arange(P)[:, None]
    il = np.arange(512)[None, :]
    for k in range(4):
        tri[:, k * 512:(k + 1) * 512] = np.where(il >= 128 * k + pidx,
                                                 0.0, NEG)
    tri[NN:, 4 * 512:5 * 512] = NEG
    blob[R["tri"]:R["tri"] + P, :] = tri[:, 0:1024]
    blob[R["tri"] + P:R["tri"] + 2 * P, :] = tri[:, 1024:2048]
    blob[R["tri"] + 2 * P:R["tri"] + 3 * P, 0:512] = tri[:, 2048:2560]

    nk = np.asarray(null_kv[0]).T                       # [64, 2]
    blob[R["misc"]:R["misc"] + DH, MC_KTAIL:MC_KTAIL + NN] = nk
    blob[R["misc"] + DH:R["misc"] + P, MC_KTAIL:MC_KTAIL + NN] = nk
    blob[R["misc"]:R["misc"] + P,
         MC_IDENT:MC_IDENT + P] = np.eye(P, dtype=F32)
    blob[R["misc"]:R["misc"] + NN, MC_VTAIL:MC_VTAIL + DH] = \
        np.asarray(null_kv[1])
    blob[R["misc"]:R["misc"] + NN, MC_VTAIL + DH:MC_VTAIL + DH + NN] = 1.0
    for p in range(PC):
        blob[R["misc"]:R["misc"] + P, MC_QB + p] = bq[p * 128:(p + 1) * 128]
    blob[R["misc"]:R["misc"] + P, MC_KB] = np.concatenate([bk, bk])
    blob[R["misc"]:R["misc"] + DH, MC_VB] = bv

    if not mask_trivial:
        for i in range(nb):
            mrow = np.where(np.asarray(mask[core * nb + i]), 0.0, NEG)
            mb = np.zeros((P, NT * 512), F32)
            for jj in range(NT):
                mb[:, jj * 512:(jj + 1) * 512] = \
                    mrow[jj * P:(jj + 1) * P][:, None]
            for band in range(4):
                blob[R["mb"] + (i * 4 + band) * P:
                     R["mb"] + (i * 4 + band + 1) * P, :] = \
                    mb[:, band * 1024:(band + 1) * 1024]

    return {"blob": blob}


def _run(x, mask, freqs, ln_g, ln_b, W_q, W_kv, W_out, null_kv, **spmd_kwargs):
    x = np.asarray(x, F32)
    mask = np.asarray(mask)
    freqs = np.asarray(freqs, F32)
    ln_g = np.asarray(ln_g, np.float64)
    ln_b = np.asarray(ln_b, np.float64)
    W_q = np.asarray(W_q, np.float64)
    W_kv = np.asarray(W_kv, np.float64)
    W_out = np.asarray(W_out, np.float64)
    null_kv = np.asarray(null_kv, F32)

    mask_trivial = bool(mask.all())
    nc = _get_program(mask_trivial)
    in_maps = [
        _host_prep(c, x, mask, freqs, ln_g, ln_b, W_q, W_kv, W_out, null_kv,
                   mask_trivial)
        for c in range(NCORES)
    ]
    res = bass_utils.run_bass_kernel_spmd(nc, in_maps, list(range(NCORES)),
                                          **spmd_kwargs)
    out = np.empty((B, N, DIM), F32)
    for b in range(B):
        out[b] = res.results[b // NB]["out"][(b % NB) * N:(b % NB + 1) * N]
    return out, res


def kernel(x, mask, freqs, ln_g, ln_b, W_q, W_kv, W_out, null_kv):
    out, _ = _run(x, mask, freqs, ln_g, ln_b, W_q, W_kv, W_out, null_kv)
    return out


# revision 12
# speedup vs baseline: 15.8086x; 4.1982x over previous
"""Trainium2 Bass kernel for nn_Attention (LN -> QKV proj -> partial RoPE ->
null-KV prepend -> causal MQA attention -> out proj).

Dispatch-cost-aware sharding: the axon PJRT path costs ~10ms fixed +
~0.9ms/core + ~0.8ms/buffer per exec, with payload bytes nearly free.
So: NCORES cores (default 2), each computing NB=4//NCORES full batches
(all 16 heads), with ONE packed f32 input blob + ONE f32 output tensor
per core. Output is a disjoint batch stack (no host reduction).

RoPE is applied post-projection via stream_shuffle partition rotation of
the biased q/k/v rows (rot contribution = shuffle * signed-sin + q * cos),
so no separate rot-weight projections are needed.

All compute ops keep uniform start-partitions (walrus checkSBSameStartPartition):
- k is projected twice (rows 0:64 and 64:128) so odd heads' QK matmuls run with
  lhsT/rhs both at base 64.
- rope groups live at rows base+(0:32) for base in {0, 64}; shuffles and
  combines stay within one base.
"""

import sys

for _p in ("/opt/trn_rl_repo",):
    if _p not in sys.path:
        sys.path.insert(0, _p)

import numpy as np

import concourse.bass as bass
import concourse.tile as tile
from concourse import bacc, mybir
from concourse import bass_utils

F32 = np.float32

B, N, DIM = 4, 1024, 1024
HEADS, DH = 16, 64
PC = HEADS // 2             # 8 head-pair groups, all on one core
ROT = 32
NN = 2                      # null kv
EPS = 1e-5
P = 128
NEG = -1.0e38
SCALE = DH ** -0.5
NT = N // P                 # 8 i-tiles / D-chunks
IB = N // 512               # 2 i-blocks

NB = 4                      # batches per core
NCORES = B // NB

dt = mybir.dt

ROT_SHUF = list(range(16, 32)) + list(range(0, 16))


def _chunks_for_block(b0):
    """j-tile chunks per i-block: lists of seq j-tile indices; 'T' = tail."""
    if b0 == 0:
        return [[0, 1], [2, 3], ["T"]]
    return [[0, 1], [2, 3], [4, 5], [6, 7], ["T"]]


def _prime_act_tables(arch):
    """Make Exp/Ln resolve to the single set containing both, so the
    act-table insertion pass emits one load instead of thrashing."""
    import concourse.hw_specs as hw_specs
    AF = mybir.ActivationFunctionType
    tables = hw_specs.get_activation_tables(arch)
    if "natural_log_exp_and_others" in tables:
        for name, fns in tables.items():
            if name != "natural_log_exp_and_others":
                fns.discard(AF.Exp)
                fns.discard(AF.Ln)


def _blob_rows(nb, mask_trivial):
    """Row offsets of each section in the packed [R, 1024] f32 blob."""
    off = {}
    r = 0
    off["x"] = r; r += nb * N
    off["wq"] = r; r += DIM          # [1024, 1024]
    off["wkv"] = r; r += DIM         # cols 0:128 = [Wk|Wk], 128:192 = Wv
    off["wout"] = r; r += DIM        # [1024, 1024]
    off["cos"] = r; r += P           # [128, 1024]
    off["sinm"] = r; r += P          # signed sin, [128, 1024]
    off["tri"] = r; r += 3 * P       # [128,2560] as 3 bands (1024,1024,512)
    off["misc"] = r; r += P          # ktail|ident|vtail|qb|kb|vb
    if not mask_trivial:
        off["mb"] = r; r += nb * 4 * P   # per-batch [128, 4096] as 4 bands
    off["_total"] = r
    return off


# misc band column layout
MC_KTAIL = 0          # [128, 128]
MC_IDENT = 128        # [128, 128]
MC_VTAIL = 256        # [128, 66]
MC_QB = 322           # [128, 8] f32
MC_KB = 330           # [128, 1] f32
MC_VB = 331           # [64, 1] f32


def _build_program(nb, mask_trivial):
    nc = bacc.Bacc("TRN2", target_bir_lowering=False, debug=False)
    _prime_act_tables(nc.m.arch)

    f32, bf16 = dt.float32, dt.bfloat16
    AF = mybir.ActivationFunctionType
    OP = mybir.AluOpType

    R = _blob_rows(nb, mask_trivial)
    d_blob = nc.dram_tensor("blob", [R["_total"], 1024], f32,
                            kind="ExternalInput")
    d_out = nc.dram_tensor("out", [nb * N, DIM], f32, kind="ExternalOutput")

    def bap(key, r0, r1, c0, c1):
        return d_blob.ap()[R[key] + r0: R[key] + r1, c0:c1]

    with tile.TileContext(nc) as tc:
        from contextlib import ExitStack

        ctx = ExitStack()
        with ctx:
            consts = ctx.enter_context(tc.tile_pool(name="consts", bufs=1))
            persist = ctx.enter_context(tc.tile_pool(name="persist", bufs=1))

            # ---- persistent SBUF tensors ----
            wq_sb = consts.tile([P, NT * 1024], bf16)      # 8 chunks x [128,1024]
            wkk_sb = consts.tile([P, NT * 128], bf16)
            wv_sb = consts.tile([P, NT * 64], bf16)
            wout_sb = consts.tile([P, PC * DIM], bf16)     # 8 pair chunks
            cos_sb = consts.tile([P, N], bf16)
            sinm_sb = consts.tile([P, N], bf16)
            tri_sb = consts.tile([P, 5 * 512], bf16)
            ktail_sb = consts.tile([P, P], bf16)
            vtail_sb = consts.tile([P, DH + 2], bf16)
            ident = consts.tile([P, P], bf16)
            qb_sb = consts.tile([P, PC], f32)
            kb_sb = consts.tile([P, 1], f32)
            vb_sb = consts.tile([DH, 1], f32)
            mb_sb = None
            if not mask_trivial:
                mb_sb = persist.tile([P, NT * 512], bf16)

            qp = persist.tile([P, PC * N], bf16)           # q pairs [128, i]
            kT = persist.tile([P, N], bf16)                # k duplicated rows
            vT = persist.tile([DH, N], bf16)
            vext = persist.tile([P, 9 * (DH + 2)], bf16)   # v + dual ones cols
            ao = persist.tile([P, PC * N], bf16)           # attn out pairs

            # ---- load + convert weights (one blob -> bf16 SBUF consts) ----
            with tc.tile_pool(name="wstg", bufs=3) as stg:
                def load_conv(dst, r0, w, tag="stg"):
                    st = stg.tile([P, DIM], f32, tag=tag, name=tag)
                    nc.sync.dma_start(st[:, 0:w],
                                      d_blob.ap()[r0:r0 + P, 0:w])
                    nc.vector.tensor_copy(dst, st[:, 0:w])

                for c in range(NT):
                    load_conv(wq_sb[:, c * 1024:(c + 1) * 1024],
                              R["wq"] + c * P, 1024)
                for c in range(NT):
                    st = stg.tile([P, DIM], f32, tag="stg", name="stg")
                    nc.sync.dma_start(st[:, 0:192],
                                      bap("wkv", c * P, (c + 1) * P, 0, 192))
                    nc.vector.tensor_copy(wkk_sb[:, c * 128:(c + 1) * 128],
                                          st[:, 0:128])
                    nc.vector.tensor_copy(wv_sb[:, c * 64:(c + 1) * 64],
                                          st[:, 128:192])
                for p in range(PC):
                    load_conv(wout_sb[:, p * DIM:(p + 1) * DIM],
                              R["wout"] + p * P, 1024)
                load_conv(cos_sb[:], R["cos"], 1024)
                load_conv(sinm_sb[:], R["sinm"], 1024)
                load_conv(tri_sb[:, 0:1024], R["tri"], 1024)
                load_conv(tri_sb[:, 1024:2048], R["tri"] + P, 1024)
                load_conv(tri_sb[:, 2048:2560], R["tri"] + 2 * P, 512)
                st = stg.tile([P, DIM], f32, tag="stg", name="stg")
                nc.sync.dma_start(st[:, 0:MC_VTAIL + DH + 2],
                                  bap("misc", 0, P, 0, MC_VTAIL + DH + 2))
                nc.vector.tensor_copy(ktail_sb[:], st[:, MC_KTAIL:MC_KTAIL + P])
                nc.vector.tensor_copy(ident[:], st[:, MC_IDENT:MC_IDENT + P])
                nc.vector.tensor_copy(vtail_sb[:],
                                      st[:, MC_VTAIL:MC_VTAIL + DH + 2])
                nc.sync.dma_start(qb_sb[:], bap("misc", 0, P, MC_QB, MC_QB + PC))
                nc.sync.dma_start(kb_sb[:], bap("misc", 0, P, MC_KB, MC_KB + 1))
                nc.sync.dma_start(vb_sb[:], bap("misc", 0, DH, MC_VB, MC_VB + 1))

            # ---- helpers (same structure as 8-head version, PC=8) ----
            def ln_reduce_tile(ph1, t, xt, rsums, accs):
                c4 = t % 4
                nc.vector.tensor_reduce(rsums[:, c4:c4 + 1], xt[:],
                                        axis=mybir.AxisListType.X, op=OP.add)
                sq = ph1.tile([P, DIM], bf16, tag="sq", name="sq")
                nc.scalar.activation(sq[:], xt[:], AF.Square,
                                     accum_out=accs[:, c4:c4 + 1])

            def ln_stats_batch(stp, rsums, accs):
                mean = stp.tile([P, 4], f32, tag="stb", name="mean")
                nc.vector.tensor_scalar(out=mean[:], in0=rsums[:],
                                        scalar1=1.0 / DIM, scalar2=None,
                                        op0=OP.mult)
                ex2 = stp.tile([P, 4], f32, tag="stb", name="ex2")
                nc.vector.tensor_scalar(out=ex2[:], in0=accs[:],
                                        scalar1=1.0 / DIM, scalar2=None,
                                        op0=OP.mult)
                var = stp.tile([P, 4], f32, tag="stb", name="var")
                nc.vector.scalar_tensor_tensor(
                    out=var[:], in0=mean[:], scalar=-1.0, in1=mean[:],
                    op0=OP.mult, op1=OP.mult)
                nc.vector.scalar_tensor_tensor(
                    out=var[:], in0=ex2[:], scalar=EPS, in1=var[:],
                    op0=OP.add, op1=OP.add)
                nc.scalar.activation(var[:], var[:], AF.Ln)
                rstd = stp.tile([P, 4], f32, tag="stb", name="rstd")
                nc.scalar.activation(rstd[:], var[:], AF.Exp, scale=-0.5)
                negmr = stp.tile([P, 4], f32, tag="stb", name="negmr")
                nc.vector.scalar_tensor_tensor(
                    out=negmr[:], in0=mean[:], scalar=-1.0, in1=rstd[:],
                    op0=OP.mult, op1=OP.mult)
                return rstd, negmr

            def ln_xn_tile(xnT, ph1, ps1, t, xt, rstd, negmr):
                c4 = t % 4
                xn = ph1.tile([P, DIM], bf16, tag="xn", name="xn")
                nc.vector.tensor_scalar(out=xn[:], in0=xt[:],
                                        scalar1=rstd[:, c4:c4 + 1],
                                        scalar2=negmr[:, c4:c4 + 1],
                                        op0=OP.mult, op1=OP.add)
                for g in range(2):
                    pst = ps1.tile([P, 512], bf16, tag="tp", name="pst")
                    for c4b in range(4):
                        c = g * 4 + c4b
                        nc.tensor.transpose(pst[:, c4b * P:(c4b + 1) * P],
                                            xn[:, c * P:(c + 1) * P], ident[:])
                    dest = xnT[:].rearrange("p (c i) -> p c i", c=NT)[
                        :, g * 4:(g + 1) * 4, t * P:(t + 1) * P]
                    src = pst[:].rearrange("p (c i) -> p c i", c=4)
                    nc.vector.tensor_copy(dest, src)

            def mm_proj(xnT, ps2, w_sb, wwidth, col0, cols, ib, rows=P):
                ps = ps2.tile([P, 512], f32, tag="proj", name="ps")
                for c in range(NT):
                    nc.tensor.matmul(
                        ps[0:rows, :],
                        w_sb[:, c * wwidth + col0: c * wwidth + col0 + cols],
                        xnT[:, c * N + ib * 512: c * N + ib * 512 + 512],
                        start=(c == 0), stop=(c == NT - 1))
                return ps

            def rope_rows(rp, dst, base, isl_c, sin_cols):
                """dst rows base:base+32 (cols isl_c slice of width 512):
                dst = dst*cos + shuffle(dst)*sinm."""
                rsl = slice(base, base + ROT)
                tmp = rp.tile([P, 512], bf16, tag="rt", name="rt")
                nc.vector.stream_shuffle(tmp[rsl, :], dst[rsl, isl_c], ROT_SHUF)
                nc.vector.tensor_tensor(out=dst[rsl, isl_c], in0=dst[rsl, isl_c],
                                        in1=cos_sb[rsl, sin_cols], op=OP.mult)
                nc.vector.tensor_tensor(out=tmp[rsl, :], in0=tmp[rsl, :],
                                        in1=sinm_sb[rsl, sin_cols], op=OP.mult)
                nc.vector.tensor_tensor(out=dst[rsl, isl_c], in0=dst[rsl, isl_c],
                                        in1=tmp[rsl, :], op=OP.add)

            def proj_ib(xnT, ps2, vtp, rp, ib):
                isl = slice(ib * 512, (ib + 1) * 512)
                for p in range(PC):
                    csl = slice(p * N + ib * 512, p * N + ib * 512 + 512)
                    ps = mm_proj(xnT, ps2, wq_sb, 1024, p * P, P, ib)
                    nc.vector.tensor_scalar(
                        out=qp[:, csl], in0=ps[:], scalar1=qb_sb[:, p:p + 1],
                        scalar2=None, op0=OP.add)
                    for base in (0, DH):
                        rope_rows(rp, qp, base, csl, isl)
                ps = mm_proj(xnT, ps2, wkk_sb, 128, 0, P, ib)
                nc.vector.tensor_scalar(out=kT[:, isl], in0=ps[:],
                                        scalar1=kb_sb[:], scalar2=None,
                                        op0=OP.add)
                for base in (0, DH):
                    rope_rows(rp, kT, base, isl, isl)
                ps = mm_proj(xnT, ps2, wv_sb, 64, 0, DH, ib, rows=DH)
                nc.vector.tensor_scalar(out=vT[:, isl], in0=ps[0:DH, :],
                                        scalar1=vb_sb[:], scalar2=None,
                                        op0=OP.add)
                rope_rows(rp, vT, 0, isl, isl)
                # v row-major + dual ones cols for this i-block's j-tiles
                for jj in range(ib * 4, ib * 4 + 4):
                    pv = vtp.tile([P, DH], bf16, tag="vt", name="pv")
                    nc.tensor.transpose(pv[:], vT[:, jj * P:(jj + 1) * P],
                                        ident[0:DH, 0:DH])
                    vbase = jj * (DH + 2)
                    nc.vector.tensor_copy(vext[:, vbase:vbase + DH], pv[:])
                    nc.vector.memset(vext[:, vbase + DH:vbase + DH + 2], 1.0)

            # ================= per-batch pipeline =================
            for b in range(nb):
                if not mask_trivial:
                    with tc.tile_pool(name="mstg", bufs=2) as mstg:
                        for band in range(4):
                            st = mstg.tile([P, DIM], f32, tag="ms", name="ms")
                            nc.sync.dma_start(
                                st[:], bap("mb", (b * 4 + band) * P,
                                           (b * 4 + band + 1) * P, 0, 1024))
                            nc.vector.tensor_copy(
                                mb_sb[:, band * 1024:(band + 1) * 1024], st[:])

                # ---- Phases 1+2: LN + projections + rope ----
                with tc.tile_pool(name="ph1sb", bufs=4) as ph1, \
                     tc.tile_pool(name="ph1st", bufs=32) as stp, \
                     tc.tile_pool(name="xnp", bufs=1) as xnp, \
                     tc.tile_pool(name="ph1ps", bufs=2, space="PSUM") as ps1, \
                     tc.tile_pool(name="ph2ps", bufs=5, space="PSUM") as ps2, \
                     tc.tile_pool(name="rope", bufs=4) as rp, \
                     tc.tile_pool(name="vtp", bufs=1, space="PSUM") as vtp:
                    xnT = xnp.tile([P, NT * N], bf16, tag="xnT", name="xnT")
                    xts = []
                    for t in range(NT):
                        xt = ph1.tile([P, DIM], f32, tag=f"x{t % 4}",
                                      name=f"xt{t}", bufs=2)
                        nc.sync.dma_start(
                            xt[:], bap("x", b * N + t * P, b * N + (t + 1) * P,
                                       0, 1024))
                        xts.append(xt)
                    for half in range(2):
                        rsums = stp.tile([P, 4], f32, tag=f"rs{half}",
                                         name=f"rsums{half}", bufs=1)
                        accs = stp.tile([P, 4], f32, tag=f"ac{half}",
                                        name=f"accs{half}", bufs=1)
                        for t in range(half * 4, half * 4 + 4):
                            ln_reduce_tile(ph1, t, xts[t], rsums, accs)
                        rstd, negmr = ln_stats_batch(stp, rsums, accs)
                        for t in range(half * 4, half * 4 + 4):
                            ln_xn_tile(xnT, ph1, ps1, t, xts[t], rstd, negmr)
                        proj_ib(xnT, ps2, vtp, rp, half)
                    nc.vector.tensor_copy(vext[:, 8 * (DH + 2):9 * (DH + 2)],
                                          vtail_sb[:])

                # ---- Phase 3: attention (pair-packed) ----
                with tc.tile_pool(name="simps", bufs=3, space="PSUM") as simps, \
                     tc.tile_pool(name="outps", bufs=1, space="PSUM") as outps, \
                     tc.tile_pool(name="atsb", bufs=6) as atsb, \
                     tc.tile_pool(name="nrm", bufs=3) as nrm:
                    for pc in range(PC):
                        rsb = nrm.tile([P, N], f32, name="rsb", tag="rsb")
                        nc.vector.memset(rsb[DH:DH + ROT, :], 1.0)
                        aots = {}
                        for b0 in range(IB):
                            chunks = _chunks_for_block(b0)
                            alljj = [jj for ch in chunks for jj in ch]
                            qhs = {}
                            psos = {}
                            for e in (0, 1):
                                hb = e * DH
                                qhs[e] = qp[hb:hb + DH,
                                            pc * N + b0 * 512:
                                            pc * N + b0 * 512 + 512]
                                psos[e] = outps.tile([P, 512], f32,
                                                     name=f"pso{e}",
                                                     tag=f"outT{e}")
                            first_av = True
                            for ch in chunks:
                                w = len(ch) * 512
                                pss = {}
                                for e in (0, 1):
                                    pss[e] = simps.tile([P, 1024], f32,
                                                        name=f"pss{e}",
                                                        tag="sim")
                                for idx, jj in enumerate(ch):
                                    for e in (0, 1):
                                        hb = e * DH
                                        seg = pss[e][:, idx * 512:(idx + 1) * 512]
                                        diag = jj != "T" and jj >= 4 * b0
                                        extra = (1 if jj == "T" else
                                                 (1 if diag else 0)
                                                 + (0 if mask_trivial else 1))
                                        if jj == "T":
                                            nc.tensor.matmul(
                                                seg, ktail_sb[hb:hb + DH, :],
                                                qhs[e], start=True, stop=False)
                                        else:
                                            nc.tensor.matmul(
                                                seg,
                                                kT[hb:hb + DH,
                                                   jj * P:(jj + 1) * P],
                                                qhs[e], start=True,
                                                stop=(extra == 0))
                                for idx, jj in enumerate(ch):
                                    for e in (0, 1):
                                        seg = pss[e][:, idx * 512:(idx + 1) * 512]
                                        if jj == "T":
                                            nc.tensor.matmul(
                                                seg, ident[:],
                                                tri_sb[:, 4 * 512:5 * 512],
                                                start=False, stop=True)
                                            continue
                                        diag = jj >= 4 * b0
                                        extra = ((1 if diag else 0)
                                                 + (0 if mask_trivial else 1))
                                        if diag:
                                            k = jj - 4 * b0
                                            extra -= 1
                                            nc.tensor.matmul(
                                                seg, ident[:],
                                                tri_sb[:, k * 512:(k + 1) * 512],
                                                start=False, stop=(extra == 0))
                                        if not mask_trivial:
                                            extra -= 1
                                            nc.tensor.matmul(
                                                seg, ident[:],
                                                mb_sb[:, jj * 512:(jj + 1) * 512],
                                                start=False, stop=(extra == 0))
                                ats = {}
                                for e in (0, 1):
                                    at = atsb.tile([P, 1024], bf16,
                                                   name=f"at{e}", tag=f"at{e}")
                                    nc.scalar.activation(at[:, 0:w],
                                                         pss[e][:, 0:w],
                                                         AF.Exp, scale=SCALE)
                                    ats[e] = at
                                for idx, jj in enumerate(ch):
                                    vjj = 8 if jj == "T" else jj
                                    vcols = vext[:, vjj * (DH + 2):
                                                 (vjj + 1) * (DH + 2)]
                                    for e in (0, 1):
                                        nc.tensor.matmul(
                                            psos[e][0:DH + 2, :], vcols,
                                            ats[e][:, idx * 512:(idx + 1) * 512],
                                            start=first_av,
                                            stop=(jj == alljj[-1]))
                                    first_av = False
                            bsl0 = slice(b0 * 512, (b0 + 1) * 512)
                            for e in (1, 0):
                                aot = nrm.tile([DH + 2, 512], f32,
                                               name=f"aot{b0}{e}",
                                               tag=f"aot{b0}{e}")
                                nc.vector.tensor_copy(aot[:],
                                                      psos[e][0:DH + 2, :])
                                if e == 1:
                                    nc.vector.tensor_copy(rsb[DH:DH + 2, bsl0],
                                                          aot[DH:DH + 2, :])
                                else:
                                    nc.vector.tensor_copy(rsb[DH:DH + 1, bsl0],
                                                          aot[DH:DH + 1, :])
                                aots[(b0, e)] = aot
                        rows2 = rsb[DH:DH + 2, :]
                        nc.scalar.activation(rows2, rows2, AF.Ln)
                        nc.scalar.activation(rows2, rows2, AF.Exp, scale=-1.0)
                        for e in (0, 1):
                            bc = nrm.tile([P, N], f32, name=f"bc{e}",
                                          tag=f"bc{e}")
                            nc.vector.stream_shuffle(bc[DH:DH + ROT, :],
                                                     rsb[DH:DH + ROT, :],
                                                     [e] * 32)
                            nc.sync.dma_start(bc[0:ROT, :], bc[DH:DH + ROT, :])
                            nc.sync.dma_start(bc[ROT:DH, :], bc[0:ROT, :])
                            for b0 in range(IB):
                                osl = slice(pc * N + b0 * 512,
                                            pc * N + b0 * 512 + 512)
                                bsl = slice(b0 * 512, (b0 + 1) * 512)
                                src = aots[(b0, e)]
                                if e == 0:
                                    nc.gpsimd.tensor_tensor(
                                        out=ao[0:DH, osl], in0=src[0:DH, :],
                                        in1=bc[0:DH, bsl], op=OP.mult)
                                else:
                                    tmp = nrm.tile([DH, 512], bf16,
                                                   name="tmpn", tag="tmpn")
                                    nc.gpsimd.tensor_tensor(
                                        out=tmp[:], in0=src[0:DH, :],
                                        in1=bc[0:DH, bsl], op=OP.mult)
                                    nc.sync.dma_start(ao[DH:P, osl], tmp[:])

                # ---- Phase 4: out projection ----
                with tc.tile_pool(name="opps", bufs=4, space="PSUM") as opps, \
                     tc.tile_pool(name="opsb", bufs=3) as opsb:
                    for t in range(NT):
                        orow = opsb.tile([P, DIM], f32, tag="orow")
                        for nb2 in range(2):
                            ps = opps.tile([P, 512], f32, tag="op")
                            for p in range(PC):
                                nc.tensor.matmul(
                                    ps[:],
                                    ao[:, p * N + t * P: p * N + t * P + 128],
                                    wout_sb[:, p * DIM + nb2 * 512:
                                            p * DIM + nb2 * 512 + 512],
                                    start=(p == 0), stop=(p == PC - 1))
                            nc.vector.tensor_copy(
                                orow[:, nb2 * 512:(nb2 + 1) * 512], ps[:])
                        nc.sync.dma_start(
                            d_out.ap()[b * N + t * P: b * N + (t + 1) * P, :],
                            orow[:])

    nc.compile()
    return nc


_PROG_CACHE = {}


def _get_program(mask_trivial, nb=NB):
    key = (nb, bool(mask_trivial))
    if key not in _PROG_CACHE:
        _PROG_CACHE[key] = _build_program(nb, key[1])
    return _PROG_CACHE[key]


def _host_prep(core, x, mask, freqs, ln_g, ln_b, W_q, W_kv, W_out, null_kv,
               mask_trivial, nb=NB):
    R = _blob_rows(nb, mask_trivial)
    blob = np.zeros((R["_total"], 1024), F32)

    for i in range(nb):
        blob[R["x"] + i * N: R["x"] + (i + 1) * N, :] = x[core * nb + i]

    Wq_eff = W_q * ln_g[:, None]                        # [1024, 1024]
    Wkv_eff = W_kv * ln_g[:, None]                      # [1024, 128]
    bq = ln_b @ W_q                                     # [1024]
    bkv = ln_b @ W_kv                                   # [128]
    Wk, Wv = Wkv_eff[:, 0:DH], Wkv_eff[:, DH:2 * DH]
    bk, bv = bkv[0:DH], bkv[DH:2 * DH]

    blob[R["wq"]:R["wq"] + DIM, :] = Wq_eff
    blob[R["wkv"]:R["wkv"] + DIM, 0:DH] = Wk
    blob[R["wkv"]:R["wkv"] + DIM, DH:2 * DH] = Wk
    blob[R["wkv"]:R["wkv"] + DIM, 128:192] = Wv
    blob[R["wout"]:R["wout"] + DIM, :] = W_out

    f = np.asarray(freqs, np.float64)                   # [1024, 32]
    blob[R["cos"]:R["cos"] + P, :] = np.tile(np.cos(f).T, (4, 1))
    s = np.sin(f).T                                     # [32, 1024]
    sm = s.copy()
    sm[0:ROT // 2, :] = -s[0:ROT // 2, :]
    blob[R["sinm"]:R["sinm"] + P, :] = np.tile(sm, (4, 1))

    tri = np.zeros((P, 5 * 512), F32)
    pidx = np.arange(P)[:, None]
    il = np.arange(512)[None, :]
    for k in range(4):
        tri[:, k * 512:(k + 1) * 512] = np.where(il >= 128 * k + pidx,
                                                 0.0, NEG)
    tri[NN:, 4 * 512:5 * 512] = NEG
    blob[R["tri"]:R["tri"] + P, :] = tri[:, 0:1024]
    blob[R["tri"] + P:R["tri"] + 2 * P, :] = tri[:, 1024:2048]
    blob[R["tri"] + 2 * P:R["tri"] + 3 * P, 0:512] = tri[:, 2048:2560]

    nk = np.asarray(null_kv[0]).T                       # [64, 2]
    blob[R["misc"]:R["misc"] + DH, MC_KTAIL:MC_KTAIL + NN] = nk
    blob[R["misc"] + DH:R["misc"] + P, MC_KTAIL:MC_KTAIL + NN] = nk
    blob[R["misc"]:R["misc"] + P,
         MC_IDENT:MC_IDENT + P] = np.eye(P, dtype=F32)
    blob[R["misc"]:R["misc"] + NN, MC_VTAIL:MC_VTAIL + DH] = \
        np.asarray(null_kv[1])
    blob[R["misc"]:R["misc"] + NN, MC_VTAIL + DH:MC_VTAIL + DH + NN] = 1.0
    for p in range(PC):
        blob[R["misc"]:R["misc"] + P, MC_QB + p] = bq[p * 128:(p + 1) * 128]
    blob[R["misc"]:R["misc"] + P, MC_KB] = np.concatenate([bk, bk])
    blob[R["misc"]:R["misc"] + DH, MC_VB] = bv

    if not mask_trivial:
        for i in range(nb):
            mrow = np.where(np.asarray(mask[core * nb + i]), 0.0, NEG)
            mb = np.zeros((P, NT * 512), F32)
            for jj in range(NT):
                mb[:, jj * 512:(jj + 1) * 512] = \
                    mrow[jj * P:(jj + 1) * P][:, None]
            for band in range(4):
                blob[R["mb"] + (i * 4 + band) * P:
                     R["mb"] + (i * 4 + band + 1) * P, :] = \
                    mb[:, band * 1024:(band + 1) * 1024]

    return {"blob": blob}


def _run(x, mask, freqs, ln_g, ln_b, W_q, W_kv, W_out, null_kv, **spmd_kwargs):
    x = np.asarray(x, F32)
    mask = np.asarray(mask)
    freqs = np.asarray(freqs, F32)
    ln_g = np.asarray(ln_g, np.float64)
    ln_b = np.asarray(ln_b, np.float64)
    W_q = np.asarray(W_q, np.float64)
    W_kv = np.asarray(W_kv, np.float64)
    W_out = np.asarray(W_out, np.float64)
    null_kv = np.asarray(null_kv, F32)

    mask_trivial = bool(mask.all())
    nc = _get_program(mask_trivial)
    in_maps = [
        _host_prep(c, x, mask, freqs, ln_g, ln_b, W_q, W_kv, W_out, null_kv,
                   mask_trivial)
        for c in range(NCORES)
    ]
    res = bass_utils.run_bass_kernel_spmd(nc, in_maps, list(range(NCORES)),
                                          **spmd_kwargs)
    out = np.empty((B, N, DIM), F32)
    for b in range(B):
        out[b] = res.results[b // NB]["out"][(b % NB) * N:(b % NB + 1) * N]
    return out, res


def kernel(x, mask, freqs, ln_g, ln_b, W_q, W_kv, W_out, null_kv):
    out, _ = _run(x, mask, freqs, ln_g, ln_b, W_q, W_kv, W_out, null_kv)
    return out


# revision 19
# speedup vs baseline: 18.6541x; 1.1800x over previous
"""Trainium2 Bass kernel for nn_Attention (LN -> QKV proj -> partial RoPE ->
null-KV prepend -> causal MQA attention -> out proj).

Dispatch-cost-aware sharding: the axon PJRT path costs ~10ms fixed +
~0.9ms/core + ~0.8ms/buffer per exec, with payload bytes nearly free.
So: NCORES cores (default 2), each computing NB=4//NCORES full batches
(all 16 heads), with ONE packed f32 input blob + ONE f32 output tensor
per core. Output is a disjoint batch stack (no host reduction).

RoPE is applied post-projection via stream_shuffle partition rotation of
the biased q/k/v rows (rot contribution = shuffle * signed-sin + q * cos),
so no separate rot-weight projections are needed.

All compute ops keep uniform start-partitions (walrus checkSBSameStartPartition):
- k is projected twice (rows 0:64 and 64:128) so odd heads' QK matmuls run with
  lhsT/rhs both at base 64.
- rope groups live at rows base+(0:32) for base in {0, 64}; shuffles and
  combines stay within one base.
"""

import sys

for _p in ("/opt/trn_rl_repo",):
    if _p not in sys.path:
        sys.path.insert(0, _p)

import numpy as np
import ml_dtypes

import concourse.bass as bass
import concourse.tile as tile
from concourse import bacc, mybir
from concourse import bass_utils

F32 = np.float32
BF16 = ml_dtypes.bfloat16

B, N, DIM = 4, 1024, 1024
HEADS, DH = 16, 64
PC = HEADS // 2             # 8 head-pair groups, all on one core
ROT = 32
NN = 2                      # null kv
EPS = 1e-5
P = 128
NEG = -1.0e38
SCALE = DH ** -0.5
NT = N // P                 # 8 i-tiles / D-chunks
IB = N // 512               # 2 i-blocks

NB = 4                      # batches per core
NCORES = B // NB

dt = mybir.dt

ROT_SHUF = list(range(16, 32)) + list(range(0, 16))


def _chunks_for_block(b0):
    """j-tile chunks per i-block: lists of seq j-tile indices; 'T' = tail."""
    if b0 == 0:
        return [[0, 1], [2, 3], ["T"]]
    return [[0, 1], [2, 3], [4, 5], [6, 7], ["T"]]


def _prime_act_tables(arch):
    """Make Exp/Ln resolve to the single set containing both, so the
    act-table insertion pass emits one load instead of thrashing."""
    import concourse.hw_specs as hw_specs
    AF = mybir.ActivationFunctionType
    tables = hw_specs.get_activation_tables(arch)
    if "natural_log_exp_and_others" in tables:
        for name, fns in tables.items():
            if name != "natural_log_exp_and_others":
                fns.discard(AF.Exp)
                fns.discard(AF.Ln)


def _blob_rows(nb, mask_trivial):
    """Row offsets of each section in the packed [R, 1024] f32 blob."""
    off = {}
    r = 0
    off["x"] = r; r += nb * N
    off["wq"] = r; r += DIM          # [1024, 1024]
    off["wkv"] = r; r += DIM         # cols 0:128 = [Wk|Wk], 128:192 = Wv
    off["wout"] = r; r += DIM        # [1024, 1024]
    off["cos"] = r; r += P           # [128, 1024]
    off["sinm"] = r; r += P          # signed sin, [128, 1024]
    off["tri"] = r; r += 3 * P       # [128,2560] as 3 bands (1024,1024,512)
    off["misc"] = r; r += P          # ktail|ident|vtail|qb|kb|vb
    if not mask_trivial:
        off["mb"] = r; r += nb * 4 * P   # per-batch [128, 4096] as 4 bands
    off["_total"] = r
    return off


# misc band column layout
MC_KTAIL = 0          # [128, 128]
MC_IDENT = 128        # [128, 128]
MC_VTAIL = 256        # [128, 66]
MC_QB = 322           # [128, 8] f32
MC_KB = 330           # [128, 1] f32
MC_VB = 331           # [64, 1] f32


def _build_program(nb, mask_trivial):
    nc = bacc.Bacc("TRN2", target_bir_lowering=False, debug=False)
    _prime_act_tables(nc.m.arch)

    f32, bf16 = dt.float32, dt.bfloat16
    AF = mybir.ActivationFunctionType
    OP = mybir.AluOpType

    R = _blob_rows(nb, mask_trivial)
    d_blob = nc.dram_tensor("blob", [R["_total"], 1024], bf16,
                            kind="ExternalInput")
    d_out = nc.dram_tensor("out", [nb * N, DIM], f32, kind="ExternalOutput")

    def bap(key, r0, r1, c0, c1):
        return d_blob.ap()[R[key] + r0: R[key] + r1, c0:c1]

    with tile.TileContext(nc) as tc:
        from contextlib import ExitStack

        ctx = ExitStack()
        with ctx:
            consts = ctx.enter_context(tc.tile_pool(name="consts", bufs=1))
            persist = ctx.enter_context(tc.tile_pool(name="persist", bufs=1))

            # ---- persistent SBUF tensors ----
            wq_sb = consts.tile([P, NT * 1024], bf16)      # 8 chunks x [128,1024]
            wkk_sb = consts.tile([P, NT * 128], bf16)
            wv_sb = consts.tile([P, NT * 64], bf16)
            wout_sb = consts.tile([P, PC * DIM], bf16)     # 8 pair chunks
            cos_sb = consts.tile([P, N], bf16)
            sinm_sb = consts.tile([P, N], bf16)
            tri_sb = consts.tile([P, 5 * 512], bf16)
            ktail_sb = consts.tile([P, P], bf16)
            vtail_sb = consts.tile([P, DH + 2], bf16)
            ident = consts.tile([P, P], bf16)
            qb_sb = consts.tile([P, PC], f32)
            kb_sb = consts.tile([P, 1], f32)
            vb_sb = consts.tile([DH, 1], f32)
            mb_sb = None
            if not mask_trivial:
                mb_sb = persist.tile([P, NT * 512], bf16)

            qp = persist.tile([P, PC * N], bf16)           # q pairs [128, i]
            kT = persist.tile([P, N], bf16)                # k duplicated rows
            vT = persist.tile([DH, N], bf16)
            vext = persist.tile([P, 9 * (DH + 2)], bf16)   # v + dual ones cols
            ao = persist.tile([P, PC * N], bf16)           # attn out pairs

            # ---- load weights (bf16 blob -> SBUF, direct DMA) ----
            with tc.tile_pool(name="wstg", bufs=2) as stg:
                for c in range(NT):
                    nc.sync.dma_start(wq_sb[:, c * 1024:(c + 1) * 1024],
                                      bap("wq", c * P, (c + 1) * P, 0, 1024))
                    nc.sync.dma_start(wkk_sb[:, c * 128:(c + 1) * 128],
                                      bap("wkv", c * P, (c + 1) * P, 0, 128))
                    nc.sync.dma_start(wv_sb[:, c * 64:(c + 1) * 64],
                                      bap("wkv", c * P, (c + 1) * P, 128, 192))
                for p in range(PC):
                    nc.sync.dma_start(wout_sb[:, p * DIM:(p + 1) * DIM],
                                      bap("wout", p * P, (p + 1) * P, 0, 1024))
                nc.sync.dma_start(cos_sb[:], bap("cos", 0, P, 0, 1024))
                nc.sync.dma_start(sinm_sb[:], bap("sinm", 0, P, 0, 1024))
                nc.sync.dma_start(tri_sb[:, 0:1024], bap("tri", 0, P, 0, 1024))
                nc.sync.dma_start(tri_sb[:, 1024:2048],
                                  bap("tri", P, 2 * P, 0, 1024))
                nc.sync.dma_start(tri_sb[:, 2048:2560],
                                  bap("tri", 2 * P, 3 * P, 0, 512))
                nc.sync.dma_start(ktail_sb[:],
                                  bap("misc", 0, P, MC_KTAIL, MC_KTAIL + P))
                nc.sync.dma_start(ident[:],
                                  bap("misc", 0, P, MC_IDENT, MC_IDENT + P))
                nc.sync.dma_start(vtail_sb[:],
                                  bap("misc", 0, P, MC_VTAIL, MC_VTAIL + DH + 2))
                bst = stg.tile([P, 16], bf16, tag="bst", name="bst")
                nc.sync.dma_start(bst[:, 0:PC],
                                  bap("misc", 0, P, MC_QB, MC_QB + PC))
                nc.sync.dma_start(bst[:, PC:PC + 1],
                                  bap("misc", 0, P, MC_KB, MC_KB + 1))
                nc.sync.dma_start(bst[0:DH, PC + 1:PC + 2],
                                  bap("misc", 0, DH, MC_VB, MC_VB + 1))
                nc.vector.tensor_copy(qb_sb[:], bst[:, 0:PC])
                nc.vector.tensor_copy(kb_sb[:], bst[:, PC:PC + 1])
                nc.vector.tensor_copy(vb_sb[:], bst[0:DH, PC + 1:PC + 2])

            # ---- helpers (same structure as 8-head version, PC=8) ----
            def ln_reduce_tile(ph1, t, xt, rsums, accs):
                c4 = t % 4
                nc.vector.tensor_reduce(rsums[:, c4:c4 + 1], xt[:],
                                        axis=mybir.AxisListType.X, op=OP.add)
                sq = ph1.tile([P, DIM], bf16, tag="sq", name="sq")
                nc.scalar.activation(sq[:], xt[:], AF.Square,
                                     accum_out=accs[:, c4:c4 + 1])

            def ln_stats_batch(stp, rsums, accs):
                mean = stp.tile([P, 4], f32, tag="stb", name="mean")
                nc.vector.tensor_scalar(out=mean[:], in0=rsums[:],
                                        scalar1=1.0 / DIM, scalar2=None,
                                        op0=OP.mult)
                ex2 = stp.tile([P, 4], f32, tag="stb", name="ex2")
                nc.vector.tensor_scalar(out=ex2[:], in0=accs[:],
                                        scalar1=1.0 / DIM, scalar2=None,
                                        op0=OP.mult)
                var = stp.tile([P, 4], f32, tag="stb", name="var")
                nc.vector.scalar_tensor_tensor(
                    out=var[:], in0=mean[:], scalar=-1.0, in1=mean[:],
                    op0=OP.mult, op1=OP.mult)
                nc.vector.scalar_tensor_tensor(
                    out=var[:], in0=ex2[:], scalar=EPS, in1=var[:],
                    op0=OP.add, op1=OP.add)
                nc.scalar.activation(var[:], var[:], AF.Ln)
                rstd = stp.tile([P, 4], f32, tag="stb", name="rstd")
                nc.scalar.activation(rstd[:], var[:], AF.Exp, scale=-0.5)
                negmr = stp.tile([P, 4], f32, tag="stb", name="negmr")
                nc.vector.scalar_tensor_tensor(
                    out=negmr[:], in0=mean[:], scalar=-1.0, in1=rstd[:],
                    op0=OP.mult, op1=OP.mult)
                return rstd, negmr

            def ln_xn_tile(xnT, ph1, ps1, t, xt, rstd, negmr):
                c4 = t % 4
                xn = ph1.tile([P, DIM], bf16, tag="xn", name="xn")
                nc.vector.tensor_scalar(out=xn[:], in0=xt[:],
                                        scalar1=rstd[:, c4:c4 + 1],
                                        scalar2=negmr[:, c4:c4 + 1],
                                        op0=OP.mult, op1=OP.add)
                for g in range(2):
                    pst = ps1.tile([P, 512], bf16, tag="tp", name="pst")
                    for c4b in range(4):
                        c = g * 4 + c4b
                        nc.tensor.transpose(pst[:, c4b * P:(c4b + 1) * P],
                                            xn[:, c * P:(c + 1) * P], ident[:])
                    dest = xnT[:].rearrange("p (c i) -> p c i", c=NT)[
                        :, g * 4:(g + 1) * 4, t * P:(t + 1) * P]
                    src = pst[:].rearrange("p (c i) -> p c i", c=4)
                    nc.vector.tensor_copy(dest, src)

            def mm_proj(xnT, ps2, w_sb, wwidth, col0, cols, ib, rows=P):
                ps = ps2.tile([P, 512], f32, tag="proj", name="ps")
                for c in range(NT):
                    nc.tensor.matmul(
                        ps[0:rows, :],
                        w_sb[:, c * wwidth + col0: c * wwidth + col0 + cols],
                        xnT[:, c * N + ib * 512: c * N + ib * 512 + 512],
                        start=(c == 0), stop=(c == NT - 1))
                return ps

            def rope_rows(rp, dst, base, isl_c, sin_cols):
                """dst rows base:base+32 (cols isl_c slice of width 512):
                dst = dst*cos + shuffle(dst)*sinm."""
                rsl = slice(base, base + ROT)
                tmp = rp.tile([P, 512], bf16, tag="rt", name="rt")
                nc.vector.stream_shuffle(tmp[rsl, :], dst[rsl, isl_c], ROT_SHUF)
                nc.vector.tensor_tensor(out=dst[rsl, isl_c], in0=dst[rsl, isl_c],
                                        in1=cos_sb[rsl, sin_cols], op=OP.mult)
                nc.vector.tensor_tensor(out=tmp[rsl, :], in0=tmp[rsl, :],
                                        in1=sinm_sb[rsl, sin_cols], op=OP.mult)
                nc.vector.tensor_tensor(out=dst[rsl, isl_c], in0=dst[rsl, isl_c],
                                        in1=tmp[rsl, :], op=OP.add)

            def proj_ib(xnT, ps2, vtp, rp, ib):
                isl = slice(ib * 512, (ib + 1) * 512)
                for p in range(PC):
                    csl = slice(p * N + ib * 512, p * N + ib * 512 + 512)
                    ps = mm_proj(xnT, ps2, wq_sb, 1024, p * P, P, ib)
                    nc.vector.tensor_scalar(
                        out=qp[:, csl], in0=ps[:], scalar1=qb_sb[:, p:p + 1],
                        scalar2=None, op0=OP.add)
                    for base in (0, DH):
                        rope_rows(rp, qp, base, csl, isl)
                ps = mm_proj(xnT, ps2, wkk_sb, 128, 0, P, ib)
                nc.vector.tensor_scalar(out=kT[:, isl], in0=ps[:],
                                        scalar1=kb_sb[:], scalar2=None,
                                        op0=OP.add)
                for base in (0, DH):
                    rope_rows(rp, kT, base, isl, isl)
                ps = mm_proj(xnT, ps2, wv_sb, 64, 0, DH, ib, rows=DH)
                nc.vector.tensor_scalar(out=vT[:, isl], in0=ps[0:DH, :],
                                        scalar1=vb_sb[:], scalar2=None,
                                        op0=OP.add)
                rope_rows(rp, vT, 0, isl, isl)
                # v row-major + dual ones cols for this i-block's j-tiles
                for jj in range(ib * 4, ib * 4 + 4):
                    pv = vtp.tile([P, DH], bf16, tag="vt", name="pv")
                    nc.tensor.transpose(pv[:], vT[:, jj * P:(jj + 1) * P],
                                        ident[0:DH, 0:DH])
                    vbase = jj * (DH + 2)
                    nc.vector.tensor_copy(vext[:, vbase:vbase + DH], pv[:])
                    nc.vector.memset(vext[:, vbase + DH:vbase + DH + 2], 1.0)

            # ================= per-batch pipeline =================
            for b in range(nb):
                if not mask_trivial:
                    for band in range(4):
                        nc.sync.dma_start(
                            mb_sb[:, band * 1024:(band + 1) * 1024],
                            bap("mb", (b * 4 + band) * P,
                                (b * 4 + band + 1) * P, 0, 1024))

                # ---- Phases 1+2: LN + projections + rope ----
                with tc.tile_pool(name="ph1sb", bufs=4) as ph1, \
                     tc.tile_pool(name="ph1st", bufs=32) as stp, \
                     tc.tile_pool(name="xnp", bufs=1) as xnp, \
                     tc.tile_pool(name="ph1ps", bufs=2, space="PSUM") as ps1, \
                     tc.tile_pool(name="ph2ps", bufs=5, space="PSUM") as ps2, \
                     tc.tile_pool(name="rope", bufs=4) as rp, \
                     tc.tile_pool(name="vtp", bufs=1, space="PSUM") as vtp:
                    xnT = xnp.tile([P, NT * N], bf16, tag="xnT", name="xnT")
                    xts = []
                    for t in range(NT):
                        xt = ph1.tile([P, DIM], bf16, tag=f"x{t % 4}",
                                      name=f"xt{t}", bufs=2)
                        nc.sync.dma_start(
                            xt[:], bap("x", b * N + t * P, b * N + (t + 1) * P,
                                       0, 1024))
                        xts.append(xt)
                    for half in range(2):
                        rsums = stp.tile([P, 4], f32, tag=f"rs{half}",
                                         name=f"rsums{half}", bufs=1)
                        accs = stp.tile([P, 4], f32, tag=f"ac{half}",
                                        name=f"accs{half}", bufs=1)
                        for t in range(half * 4, half * 4 + 4):
                            ln_reduce_tile(ph1, t, xts[t], rsums, accs)
                        rstd, negmr = ln_stats_batch(stp, rsums, accs)
                        for t in range(half * 4, half * 4 + 4):
                            ln_xn_tile(xnT, ph1, ps1, t, xts[t], rstd, negmr)
                        proj_ib(xnT, ps2, vtp, rp, half)
                    nc.vector.tensor_copy(vext[:, 8 * (DH + 2):9 * (DH + 2)],
                                          vtail_sb[:])

                # ---- Phase 3: attention (pair-packed) ----
                with tc.tile_pool(name="simps", bufs=3, space="PSUM") as simps, \
                     tc.tile_pool(name="outps", bufs=1, space="PSUM") as outps, \
                     tc.tile_pool(name="atsb", bufs=6) as atsb, \
                     tc.tile_pool(name="nrm", bufs=3) as nrm:
                    for pc in range(PC):
                        rsb = nrm.tile([P, N], f32, name="rsb", tag="rsb")
                        nc.vector.memset(rsb[DH:DH + ROT, :], 1.0)
                        aots = {}
                        for b0 in range(IB):
                            chunks = _chunks_for_block(b0)
                            alljj = [jj for ch in chunks for jj in ch]
                            qhs = {}
                            psos = {}
                            for e in (0, 1):
                                hb = e * DH
                                qhs[e] = qp[hb:hb + DH,
                                            pc * N + b0 * 512:
                                            pc * N + b0 * 512 + 512]
                                psos[e] = outps.tile([P, 512], f32,
                                                     name=f"pso{e}",
                                                     tag=f"outT{e}")
                            first_av = True
                            for ch in chunks:
                                w = len(ch) * 512
                                pss = {}
                                for e in (0, 1):
                                    pss[e] = simps.tile([P, 1024], f32,
                                                        name=f"pss{e}",
                                                        tag="sim")
                                for idx, jj in enumerate(ch):
                                    for e in (0, 1):
                                        hb = e * DH
                                        seg = pss[e][:, idx * 512:(idx + 1) * 512]
                                        diag = jj != "T" and jj >= 4 * b0
                                        extra = (1 if jj == "T" else
                                                 (1 if diag else 0)
                                                 + (0 if mask_trivial else 1))
                                        if jj == "T":
                                            nc.tensor.matmul(
                                                seg, ktail_sb[hb:hb + DH, :],
                                                qhs[e], start=True, stop=False)
                                        else:
                                            nc.tensor.matmul(
                                                seg,
                                                kT[hb:hb + DH,
                                                   jj * P:(jj + 1) * P],
                                                qhs[e], start=True,
                                                stop=(extra == 0))
                                for idx, jj in enumerate(ch):
                                    for e in (0, 1):
                                        seg = pss[e][:, idx * 512:(idx + 1) * 512]
                                        if jj == "T":
                                            nc.tensor.matmul(
                                                seg, ident[:],
                                                tri_sb[:, 4 * 512:5 * 512],
                                                start=False, stop=True)
                                            continue
                                        diag = jj >= 4 * b0
                                        extra = ((1 if diag else 0)
                                                 + (0 if mask_trivial else 1))
                                        if diag:
                                            k = jj - 4 * b0
                                            extra -= 1
                                            nc.tensor.matmul(
                                                seg, ident[:],
                                                tri_sb[:, k * 512:(k + 1) * 512],
                                                start=False, stop=(extra == 0))
                                        if not mask_trivial:
                                            extra -= 1
                                            nc.tensor.matmul(
                                                seg, ident[:],
                                                mb_sb[:, jj * 512:(jj + 1) * 512],
                                                start=False, stop=(extra == 0))
                                ats = {}
                                for e in (0, 1):
                                    at = atsb.tile([P, 1024], bf16,
                                                   name=f"at{e}", tag=f"at{e}")
                                    nc.scalar.activation(at[:, 0:w],
                                                         pss[e][:, 0:w],
                                                         AF.Exp, scale=SCALE)
                                    ats[e] = at
                                for idx, jj in enumerate(ch):
                                    vjj = 8 if jj == "T" else jj
                                    vcols = vext[:, vjj * (DH + 2):
                                                 (vjj + 1) * (DH + 2)]
                                    for e in (0, 1):
                                        nc.tensor.matmul(
                                            psos[e][0:DH + 2, :], vcols,
                                            ats[e][:, idx * 512:(idx + 1) * 512],
                                            start=first_av,
                                            stop=(jj == alljj[-1]))
                                    first_av = False
                            bsl0 = slice(b0 * 512, (b0 + 1) * 512)
                            for e in (1, 0):
                                aot = nrm.tile([DH + 2, 512], f32,
                                               name=f"aot{b0}{e}",
                                               tag=f"aot{b0}{e}")
                                nc.vector.tensor_copy(aot[:],
                                                      psos[e][0:DH + 2, :])
                                if e == 1:
                                    nc.vector.tensor_copy(rsb[DH:DH + 2, bsl0],
                                                          aot[DH:DH + 2, :])
                                else:
                                    nc.vector.tensor_copy(rsb[DH:DH + 1, bsl0],
                                                          aot[DH:DH + 1, :])
                                aots[(b0, e)] = aot
                        rows2 = rsb[DH:DH + 2, :]
                        nc.scalar.activation(rows2, rows2, AF.Ln)
                        nc.scalar.activation(rows2, rows2, AF.Exp, scale=-1.0)
                        for e in (0, 1):
                            bc = nrm.tile([P, N], f32, name=f"bc{e}",
                                          tag=f"bc{e}")
                            nc.vector.stream_shuffle(bc[DH:DH + ROT, :],
                                                     rsb[DH:DH + ROT, :],
                                                     [e] * 32)
                            nc.sync.dma_start(bc[0:ROT, :], bc[DH:DH + ROT, :])
                            nc.sync.dma_start(bc[ROT:DH, :], bc[0:ROT, :])
                            for b0 in range(IB):
                                osl = slice(pc * N + b0 * 512,
                                            pc * N + b0 * 512 + 512)
                                bsl = slice(b0 * 512, (b0 + 1) * 512)
                                src = aots[(b0, e)]
                                if e == 0:
                                    nc.gpsimd.tensor_tensor(
                                        out=ao[0:DH, osl], in0=src[0:DH, :],
                                        in1=bc[0:DH, bsl], op=OP.mult)
                                else:
                                    tmp = nrm.tile([DH, 512], bf16,
                                                   name="tmpn", tag="tmpn")
                                    nc.gpsimd.tensor_tensor(
                                        out=tmp[:], in0=src[0:DH, :],
                                        in1=bc[0:DH, bsl], op=OP.mult)
                                    nc.sync.dma_start(ao[DH:P, osl], tmp[:])

                # ---- Phase 4: out projection ----
                with tc.tile_pool(name="opps", bufs=4, space="PSUM") as opps, \
                     tc.tile_pool(name="opsb", bufs=3) as opsb:
                    for t in range(NT):
                        orow = opsb.tile([P, DIM], f32, tag="orow")
                        for nb2 in range(2):
                            ps = opps.tile([P, 512], f32, tag="op")
                            for p in range(PC):
                                nc.tensor.matmul(
                                    ps[:],
                                    ao[:, p * N + t * P: p * N + t * P + 128],
                                    wout_sb[:, p * DIM + nb2 * 512:
                                            p * DIM + nb2 * 512 + 512],
                                    start=(p == 0), stop=(p == PC - 1))
                            nc.vector.tensor_copy(
                                orow[:, nb2 * 512:(nb2 + 1) * 512], ps[:])
                        nc.sync.dma_start(
                            d_out.ap()[b * N + t * P: b * N + (t + 1) * P, :],
                            orow[:])

    nc.compile()
    return nc


_PROG_CACHE = {}


def _get_program(mask_trivial, nb=NB):
    key = (nb, bool(mask_trivial))
    if key not in _PROG_CACHE:
        _PROG_CACHE[key] = _build_program(nb, key[1])
    return _PROG_CACHE[key]


def _host_prep(core, x, mask, freqs, ln_g, ln_b, W_q, W_kv, W_out, null_kv,
               mask_trivial, nb=NB):
    R = _blob_rows(nb, mask_trivial)
    blob = np.zeros((R["_total"], 1024), BF16)

    for i in range(nb):
        blob[R["x"] + i * N: R["x"] + (i + 1) * N, :] = x[core * nb + i]

    Wq_eff = W_q * ln_g[:, None]                        # [1024, 1024]
    Wkv_eff = W_kv * ln_g[:, None]                      # [1024, 128]
    bq = ln_b @ W_q                                     # [1024]
    bkv = ln_b @ W_kv                                   # [128]
    Wk, Wv = Wkv_eff[:, 0:DH], Wkv_eff[:, DH:2 * DH]
    bk, bv = bkv[0:DH], bkv[DH:2 * DH]

    blob[R["wq"]:R["wq"] + DIM, :] = Wq_eff
    blob[R["wkv"]:R["wkv"] + DIM, 0:DH] = Wk
    blob[R["wkv"]:R["wkv"] + DIM, DH:2 * DH] = Wk
    blob[R["wkv"]:R["wkv"] + DIM, 128:192] = Wv
    blob[R["wout"]:R["wout"] + DIM, :] = W_out

    f = np.asarray(freqs, np.float64)                   # [1024, 32]
    blob[R["cos"]:R["cos"] + P, :] = np.tile(np.cos(f).T, (4, 1))
    s = np.sin(f).T                                     # [32, 1024]
    sm = s.copy()
    sm[0:ROT // 2, :] = -s[0:ROT // 2, :]
    blob[R["sinm"]:R["sinm"] + P, :] = np.tile(sm, (4, 1))

    tri = np.zeros((P, 5 * 512), F32)
    pidx = np.arange(P)[:, None]
    il = np.arange(512)[None, :]
    for k in range(4):
        tri[:, k * 512:(k + 1) * 512] = np.where(il >= 128 * k + pidx,
                                                 0.0, NEG)
    tri[NN:, 4 * 512:5 * 512] = NEG
    blob[R["tri"]:R["tri"] + P, :] = tri[:, 0:1024]
    blob[R["tri"] + P:R["tri"] + 2 * P, :] = tri[:, 1024:2048]
    blob[R["tri"] + 2 * P:R["tri"] + 3 * P, 0:512] = tri[:, 2048:2560]

    nk = np.asarray(null_kv[0]).T                       # [64, 2]
    blob[R["misc"]:R["misc"] + DH, MC_KTAIL:MC_KTAIL + NN] = nk
    blob[R["misc"] + DH:R["misc"] + P, MC_KTAIL:MC_KTAIL + NN] = nk
    blob[R["misc"]:R["misc"] + P,
         MC_IDENT:MC_IDENT + P] = np.eye(P, dtype=F32)
    blob[R["misc"]:R["misc"] + NN, MC_VTAIL:MC_VTAIL + DH] = \
        np.asarray(null_kv[1])
    blob[R["misc"]:R["misc"] + NN, MC_VTAIL + DH:MC_VTAIL + DH + NN] = 1.0
    for p in range(PC):
        blob[R["misc"]:R["misc"] + P, MC_QB + p] = bq[p * 128:(p + 1) * 128]
    blob[R["misc"]:R["misc"] + P, MC_KB] = np.concatenate([bk, bk])
    blob[R["misc"]:R["misc"] + DH, MC_VB] = bv

    if not mask_trivial:
        for i in range(nb):
            mrow = np.where(np.asarray(mask[core * nb + i]), 0.0, NEG)
            mb = np.zeros((P, NT * 512), F32)
            for jj in range(NT):
                mb[:, jj * 512:(jj + 1) * 512] = \
                    mrow[jj * P:(jj + 1) * P][:, None]
            for band in range(4):
                blob[R["mb"] + (i * 4 + band) * P:
                     R["mb"] + (i * 4 + band + 1) * P, :] = \
                    mb[:, band * 1024:(band + 1) * 1024]

    return {"blob": blob}


def _run(x, mask, freqs, ln_g, ln_b, W_q, W_kv, W_out, null_kv, **spmd_kwargs):
    x = np.asarray(x, F32)
    mask = np.asarray(mask)
    freqs = np.asarray(freqs, F32)
    ln_g = np.asarray(ln_g, np.float64)
    ln_b = np.asarray(ln_b, np.float64)
    W_q = np.asarray(W_q, np.float64)
    W_kv = np.asarray(W_kv, np.float64)
    W_out = np.asarray(W_out, np.float64)
    null_kv = np.asarray(null_kv, F32)

    mask_trivial = bool(mask.all())
    nc = _get_program(mask_trivial)
    in_maps = [
        _host_prep(c, x, mask, freqs, ln_g, ln_b, W_q, W_kv, W_out, null_kv,
                   mask_trivial)
        for c in range(NCORES)
    ]
    res = bass_utils.run_bass_kernel_spmd(nc, in_maps, list(range(NCORES)),
                                          **spmd_kwargs)
    out = np.empty((B, N, DIM), F32)
    for b in range(B):
        out[b] = res.results[b // NB]["out"][(b % NB) * N:(b % NB + 1) * N]
    return out, res


def kernel(x, mask, freqs, ln_g, ln_b, W_q, W_kv, W_out, null_kv):
    out, _ = _run(x, mask, freqs, ln_g, ln_b, W_q, W_kv, W_out, null_kv)
    return out


# revision 34
# speedup vs baseline: 23.7776x; 1.2747x over previous
"""Trainium2 Bass kernel for nn_Attention (LN -> QKV proj -> partial RoPE ->
null-KV prepend -> causal MQA attention -> out proj).

Dispatch-cost-aware sharding: the axon PJRT path costs ~10ms fixed +
~0.9ms/core + ~0.8ms/buffer per exec, with payload bytes nearly free.
So: NCORES cores (default 2), each computing NB=4//NCORES full batches
(all 16 heads), with ONE packed f32 input blob + ONE f32 output tensor
per core. Output is a disjoint batch stack (no host reduction).

RoPE is applied post-projection via stream_shuffle partition rotation of
the biased q/k/v rows (rot contribution = shuffle * signed-sin + q * cos),
so no separate rot-weight projections are needed.

All compute ops keep uniform start-partitions (walrus checkSBSameStartPartition):
- k is projected twice (rows 0:64 and 64:128) so odd heads' QK matmuls run with
  lhsT/rhs both at base 64.
- rope groups live at rows base+(0:32) for base in {0, 64}; shuffles and
  combines stay within one base.
"""

import sys

for _p in ("/opt/trn_rl_repo",):
    if _p not in sys.path:
        sys.path.insert(0, _p)

import numpy as np
import ml_dtypes

import concourse.bass as bass
import concourse.tile as tile
from concourse import bacc, mybir
from concourse import bass_utils

F32 = np.float32
BF16 = ml_dtypes.bfloat16

B, N, DIM = 4, 1024, 1024
HEADS, DH = 16, 64
PC = HEADS // 2             # 8 head-pair groups, all on one core
ROT = 32
NN = 2                      # null kv
EPS = 1e-5
P = 128
NEG = -1.0e38
SCALE = DH ** -0.5
NT = N // P                 # 8 i-tiles / D-chunks
IB = N // 512               # 2 i-blocks

NB = 4                      # batches per core
NCORES = B // NB

dt = mybir.dt

ROT_SHUF = list(range(16, 32)) + list(range(0, 16))


def _chunks_for_block(b0):
    """j-tile chunks per i-block: lists of seq j-tile indices; 'T' = tail."""
    if b0 == 0:
        return [[0, 1], [2, 3], ["T"]]
    return [[0, 1], [2, 3], [4, 5], [6, 7], ["T"]]


def _prime_act_tables(arch):
    """Make Exp/Ln resolve to the single set containing both, so the
    act-table insertion pass emits one load instead of thrashing."""
    import concourse.hw_specs as hw_specs
    AF = mybir.ActivationFunctionType
    tables = hw_specs.get_activation_tables(arch)
    if "natural_log_exp_and_others" in tables:
        for name, fns in tables.items():
            if name != "natural_log_exp_and_others":
                for f in (AF.Exp, AF.Ln, AF.Square, AF.Identity, AF.Copy):
                    fns.discard(f)


def _blob_rows(nb, mask_trivial):
    """Row offsets of each section in the packed [R, 1024] f32 blob."""
    off = {}
    r = 0
    off["x"] = r; r += nb * N
    off["wq"] = r; r += DIM          # [1024, 1024]
    off["wkv"] = r; r += DIM         # cols 0:128 = [Wk|Wk], 128:192 = Wv
    off["wout"] = r; r += DIM        # [1024, 1024]
    off["cos"] = r; r += P           # [128, 1024]
    off["sinm"] = r; r += P          # signed sin, [128, 1024]
    off["tri"] = r; r += 3 * P       # [128,2560] as 3 bands (1024,1024,512)
    off["misc"] = r; r += P          # ktail|ident|vtail|qb|kb|vb
    if not mask_trivial:
        off["mb"] = r; r += nb * 4 * P   # per-batch [128, 4096] as 4 bands
    off["_total"] = r
    return off


# misc band column layout
MC_KTAIL = 0          # [128, 128]
MC_IDENT = 128        # [128, 128]
MC_VTAIL = 256        # [128, 66]
MC_QB = 322           # [128, 8] f32
MC_KB = 330           # [128, 1] f32
MC_VB = 331           # [64, 1] f32


def _build_program(nb, mask_trivial):
    nc = bacc.Bacc("TRN2", target_bir_lowering=False, debug=False)
    _prime_act_tables(nc.m.arch)

    f32, bf16 = dt.float32, dt.bfloat16
    AF = mybir.ActivationFunctionType
    OP = mybir.AluOpType

    R = _blob_rows(nb, mask_trivial)
    d_blob = nc.dram_tensor("blob", [R["_total"], 1024], bf16,
                            kind="ExternalInput")
    d_out = nc.dram_tensor("out", [nb * N, DIM], f32, kind="ExternalOutput")

    def bap(key, r0, r1, c0, c1):
        return d_blob.ap()[R[key] + r0: R[key] + r1, c0:c1]

    with tile.TileContext(nc) as tc:
        from contextlib import ExitStack

        ctx = ExitStack()
        with ctx:
            consts = ctx.enter_context(tc.tile_pool(name="consts", bufs=1))
            persist = ctx.enter_context(tc.tile_pool(name="persist", bufs=1))

            # ---- persistent SBUF tensors ----
            wq_sb = consts.tile([P, NT * 1024], bf16)      # 8 chunks x [128,1024]
            wkk_sb = consts.tile([P, NT * 128], bf16)
            wv_sb = consts.tile([P, NT * 64], bf16)
            wout_sb = consts.tile([P, PC * DIM], bf16)     # 8 pair chunks
            cos_sb = consts.tile([P, N], bf16)
            sinm_sb = consts.tile([P, N], bf16)
            tri_sb = consts.tile([P, 5 * 512], bf16)
            ktail_sb = consts.tile([P, P], bf16)
            vtail_sb = consts.tile([P, DH + 2], bf16)
            ident = consts.tile([P, P], bf16)
            qb_sb = consts.tile([P, PC], f32)
            kb_sb = consts.tile([P, 1], f32)
            vb_sb = consts.tile([DH, 1], f32)
            mb_sb = None
            if not mask_trivial:
                mb_sb = persist.tile([P, NT * 512], bf16)

            qp = persist.tile([P, PC * N], bf16)           # q pairs [128, i]
            kT = persist.tile([P, N], bf16)                # k duplicated rows
            vT = persist.tile([DH, N], bf16)
            vext = persist.tile([P, 9 * (DH + 2)], bf16)   # v + dual ones cols
            ao = persist.tile([P, PC * N], bf16)           # attn out pairs

            # ---- load weights (bf16 blob -> SBUF, direct DMA) ----
            with tc.tile_pool(name="wstg", bufs=2) as stg:
                for c in range(NT):
                    nc.sync.dma_start(wq_sb[:, c * 1024:(c + 1) * 1024],
                                      bap("wq", c * P, (c + 1) * P, 0, 1024))
                    nc.sync.dma_start(wkk_sb[:, c * 128:(c + 1) * 128],
                                      bap("wkv", c * P, (c + 1) * P, 0, 128))
                    nc.sync.dma_start(wv_sb[:, c * 64:(c + 1) * 64],
                                      bap("wkv", c * P, (c + 1) * P, 128, 192))
                for p in range(PC):
                    nc.sync.dma_start(wout_sb[:, p * DIM:(p + 1) * DIM],
                                      bap("wout", p * P, (p + 1) * P, 0, 1024))
                nc.sync.dma_start(cos_sb[:], bap("cos", 0, P, 0, 1024))
                nc.sync.dma_start(sinm_sb[:], bap("sinm", 0, P, 0, 1024))
                nc.sync.dma_start(tri_sb[:, 0:1024], bap("tri", 0, P, 0, 1024))
                nc.sync.dma_start(tri_sb[:, 1024:2048],
                                  bap("tri", P, 2 * P, 0, 1024))
                nc.sync.dma_start(tri_sb[:, 2048:2560],
                                  bap("tri", 2 * P, 3 * P, 0, 512))
                nc.sync.dma_start(ktail_sb[:],
                                  bap("misc", 0, P, MC_KTAIL, MC_KTAIL + P))
                nc.sync.dma_start(ident[:],
                                  bap("misc", 0, P, MC_IDENT, MC_IDENT + P))
                nc.sync.dma_start(vtail_sb[:],
                                  bap("misc", 0, P, MC_VTAIL, MC_VTAIL + DH + 2))
                bst = stg.tile([P, 16], bf16, tag="bst", name="bst")
                nc.sync.dma_start(bst[:, 0:PC],
                                  bap("misc", 0, P, MC_QB, MC_QB + PC))
                nc.sync.dma_start(bst[:, PC:PC + 1],
                                  bap("misc", 0, P, MC_KB, MC_KB + 1))
                nc.sync.dma_start(bst[0:DH, PC + 1:PC + 2],
                                  bap("misc", 0, DH, MC_VB, MC_VB + 1))
                nc.vector.tensor_copy(qb_sb[:], bst[:, 0:PC])
                nc.vector.tensor_copy(kb_sb[:], bst[:, PC:PC + 1])
                nc.vector.tensor_copy(vb_sb[:], bst[0:DH, PC + 1:PC + 2])

            # ---- helpers (same structure as 8-head version, PC=8) ----
            def ln_reduce_tile(ph1, t, xt, rsums, accs):
                c4 = t % 4
                nc.vector.tensor_reduce(rsums[:, c4:c4 + 1], xt[:],
                                        axis=mybir.AxisListType.X, op=OP.add)
                sq = ph1.tile([P, DIM], bf16, tag="sq", name="sq")
                nc.scalar.activation(sq[:], xt[:], AF.Square,
                                     accum_out=accs[:, c4:c4 + 1])

            def ln_stats_batch(stp, rsums, accs):
                mean = stp.tile([P, 4], f32, tag="stb", name="mean")
                nc.vector.tensor_scalar(out=mean[:], in0=rsums[:],
                                        scalar1=1.0 / DIM, scalar2=None,
                                        op0=OP.mult)
                ex2 = stp.tile([P, 4], f32, tag="stb", name="ex2")
                nc.vector.tensor_scalar(out=ex2[:], in0=accs[:],
                                        scalar1=1.0 / DIM, scalar2=None,
                                        op0=OP.mult)
                var = stp.tile([P, 4], f32, tag="stb", name="var")
                nc.vector.scalar_tensor_tensor(
                    out=var[:], in0=mean[:], scalar=-1.0, in1=mean[:],
                    op0=OP.mult, op1=OP.mult)
                nc.vector.scalar_tensor_tensor(
                    out=var[:], in0=ex2[:], scalar=EPS, in1=var[:],
                    op0=OP.add, op1=OP.add)
                nc.scalar.activation(var[:], var[:], AF.Ln)
                rstd = stp.tile([P, 4], f32, tag="stb", name="rstd")
                nc.scalar.activation(rstd[:], var[:], AF.Exp, scale=-0.5)
                negmr = stp.tile([P, 4], f32, tag="stb", name="negmr")
                nc.vector.scalar_tensor_tensor(
                    out=negmr[:], in0=mean[:], scalar=-1.0, in1=rstd[:],
                    op0=OP.mult, op1=OP.mult)
                return rstd, negmr

            def ln_xn_tile(xnT, ph1, ps1, t, xt, rstd, negmr):
                c4 = t % 4
                xn = ph1.tile([P, DIM], bf16, tag="xn", name="xn")
                nc.vector.tensor_scalar(out=xn[:], in0=xt[:],
                                        scalar1=rstd[:, c4:c4 + 1],
                                        scalar2=negmr[:, c4:c4 + 1],
                                        op0=OP.mult, op1=OP.add)
                for g in range(2):
                    pst = ps1.tile([P, 512], bf16, tag="tp", name="pst")
                    for c4b in range(4):
                        c = g * 4 + c4b
                        nc.tensor.transpose(pst[:, c4b * P:(c4b + 1) * P],
                                            xn[:, c * P:(c + 1) * P], ident[:])
                    dest = xnT[:].rearrange("p (c i) -> p c i", c=NT)[
                        :, g * 4:(g + 1) * 4, t * P:(t + 1) * P]
                    src = pst[:].rearrange("p (c i) -> p c i", c=4)
                    nc.scalar.copy(dest, src)

            def mm_proj(xnT, ps2, w_sb, wwidth, col0, cols, ib, rows=P):
                ps = ps2.tile([P, 512], f32, tag="proj", name="ps")
                for c in range(NT):
                    nc.tensor.matmul(
                        ps[0:rows, :],
                        w_sb[:, c * wwidth + col0: c * wwidth + col0 + cols],
                        xnT[:, c * N + ib * 512: c * N + ib * 512 + 512],
                        start=(c == 0), stop=(c == NT - 1))
                return ps

            def rope_rows(rp, dst, base, isl_c, sin_cols):
                """dst rows base:base+32 (cols isl_c slice of width 512):
                dst = dst*cos + shuffle(dst)*sinm."""
                rsl = slice(base, base + ROT)
                tmp = rp.tile([P, 512], bf16, tag="rt", name="rt")
                nc.vector.stream_shuffle(tmp[rsl, :], dst[rsl, isl_c], ROT_SHUF)
                nc.vector.tensor_tensor(out=dst[rsl, isl_c],
                                        in0=dst[rsl, isl_c],
                                        in1=cos_sb[rsl, sin_cols], op=OP.mult)
                nc.vector.tensor_tensor(out=tmp[rsl, :], in0=tmp[rsl, :],
                                        in1=sinm_sb[rsl, sin_cols], op=OP.mult)
                nc.vector.tensor_tensor(out=dst[rsl, isl_c],
                                        in0=dst[rsl, isl_c],
                                        in1=tmp[rsl, :], op=OP.add)

            def proj_ib(xnT, ps2, vtp, rp, ib):
                isl = slice(ib * 512, (ib + 1) * 512)
                for p in range(PC):
                    csl = slice(p * N + ib * 512, p * N + ib * 512 + 512)
                    ps = mm_proj(xnT, ps2, wq_sb, 1024, p * P, P, ib)
                    nc.scalar.add(qp[:, csl], ps[:], qb_sb[:, p:p + 1])
                    for base in (0, DH):
                        rope_rows(rp, qp, base, csl, isl)
                ps = mm_proj(xnT, ps2, wkk_sb, 128, 0, P, ib)
                nc.scalar.add(kT[:, isl], ps[:], kb_sb[:])
                for base in (0, DH):
                    rope_rows(rp, kT, base, isl, isl)
                ps = mm_proj(xnT, ps2, wv_sb, 64, 0, DH, ib, rows=DH)
                nc.scalar.add(vT[:, isl], ps[0:DH, :], vb_sb[:])
                rope_rows(rp, vT, 0, isl, isl)
                # v row-major + dual ones cols for this i-block's j-tiles
                for jj in range(ib * 4, ib * 4 + 4):
                    pv = vtp.tile([P, DH], bf16, tag="vt", name="pv")
                    nc.tensor.transpose(pv[:], vT[:, jj * P:(jj + 1) * P],
                                        ident[0:DH, 0:DH])
                    vbase = jj * (DH + 2)
                    nc.vector.tensor_copy(vext[:, vbase:vbase + DH], pv[:])
                    nc.vector.memset(vext[:, vbase + DH:vbase + DH + 2], 1.0)

            # ================= per-batch pipeline =================
            for b in range(nb):
                if not mask_trivial:
                    for band in range(4):
                        nc.sync.dma_start(
                            mb_sb[:, band * 1024:(band + 1) * 1024],
                            bap("mb", (b * 4 + band) * P,
                                (b * 4 + band + 1) * P, 0, 1024))

                # ---- Phases 1+2: LN + projections + rope ----
                with tc.tile_pool(name="ph1sb", bufs=4) as ph1, \
                     tc.tile_pool(name="ph1st", bufs=32) as stp, \
                     tc.tile_pool(name="xnp", bufs=1) as xnp, \
                     tc.tile_pool(name="ph1ps", bufs=2, space="PSUM") as ps1, \
                     tc.tile_pool(name="ph2ps", bufs=5, space="PSUM") as ps2, \
                     tc.tile_pool(name="rope", bufs=4) as rp, \
                     tc.tile_pool(name="vtp", bufs=1, space="PSUM") as vtp:
                    xnT = xnp.tile([P, NT * N], bf16, tag="xnT", name="xnT")
                    xts = []
                    for t in range(NT):
                        xt = ph1.tile([P, DIM], bf16, tag=f"x{t % 4}",
                                      name=f"xt{t}", bufs=2)
                        nc.sync.dma_start(
                            xt[:], bap("x", b * N + t * P, b * N + (t + 1) * P,
                                       0, 1024))
                        xts.append(xt)
                    for half in range(2):
                        rsums = stp.tile([P, 4], f32, tag=f"rs{half}",
                                         name=f"rsums{half}", bufs=1)
                        accs = stp.tile([P, 4], f32, tag=f"ac{half}",
                                        name=f"accs{half}", bufs=1)
                        for t in range(half * 4, half * 4 + 4):
                            ln_reduce_tile(ph1, t, xts[t], rsums, accs)
                        rstd, negmr = ln_stats_batch(stp, rsums, accs)
                        for t in range(half * 4, half * 4 + 4):
                            ln_xn_tile(xnT, ph1, ps1, t, xts[t], rstd, negmr)
                        proj_ib(xnT, ps2, vtp, rp, half)
                    nc.vector.tensor_copy(vext[:, 8 * (DH + 2):9 * (DH + 2)],
                                          vtail_sb[:])

                # ---- Phase 3: attention (pair-packed) ----
                with tc.tile_pool(name="simps", bufs=3, space="PSUM") as simps, \
                     tc.tile_pool(name="outps", bufs=1, space="PSUM") as outps, \
                     tc.tile_pool(name="atsb", bufs=6) as atsb, \
                     tc.tile_pool(name="nrm", bufs=3) as nrm:
                    for pc in range(PC):
                        rsb = nrm.tile([P, N], f32, name="rsb", tag="rsb")
                        nc.vector.memset(rsb[DH:DH + ROT, :], 1.0)
                        aots = {}
                        for b0 in range(IB):
                            chunks = _chunks_for_block(b0)
                            alljj = [jj for ch in chunks for jj in ch]
                            qhs = {}
                            psos = {}
                            for e in (0, 1):
                                hb = e * DH
                                qhs[e] = qp[hb:hb + DH,
                                            pc * N + b0 * 512:
                                            pc * N + b0 * 512 + 512]
                                psos[e] = outps.tile([P, 512], f32,
                                                     name=f"pso{e}",
                                                     tag=f"outT{e}")
                            first_av = True
                            for ch in chunks:
                                w = len(ch) * 512
                                pss = {}
                                for e in (0, 1):
                                    pss[e] = simps.tile([P, 1024], f32,
                                                        name=f"pss{e}",
                                                        tag="sim")
                                for idx, jj in enumerate(ch):
                                    for e in (0, 1):
                                        hb = e * DH
                                        seg = pss[e][:, idx * 512:(idx + 1) * 512]
                                        diag = jj != "T" and jj >= 4 * b0
                                        extra = (1 if jj == "T" else
                                                 (1 if diag else 0)
                                                 + (0 if mask_trivial else 1))
                                        if jj == "T":
                                            nc.tensor.matmul(
                                                seg, ktail_sb[hb:hb + DH, :],
                                                qhs[e], start=True, stop=False)
                                        else:
                                            nc.tensor.matmul(
                                                seg,
                                                kT[hb:hb + DH,
                                                   jj * P:(jj + 1) * P],
                                                qhs[e], start=True,
                                                stop=(extra == 0))
                                for idx, jj in enumerate(ch):
                                    for e in (0, 1):
                                        seg = pss[e][:, idx * 512:(idx + 1) * 512]
                                        if jj == "T":
                                            nc.tensor.matmul(
                                                seg, ident[:],
                                                tri_sb[:, 4 * 512:5 * 512],
                                                start=False, stop=True)
                                            continue
                                        diag = jj >= 4 * b0
                                        extra = ((1 if diag else 0)
                                                 + (0 if mask_trivial else 1))
                                        if diag:
                                            k = jj - 4 * b0
                                            extra -= 1
                                            nc.tensor.matmul(
                                                seg, ident[:],
                                                tri_sb[:, k * 512:(k + 1) * 512],
                                                start=False, stop=(extra == 0))
                                        if not mask_trivial:
                                            extra -= 1
                                            nc.tensor.matmul(
                                                seg, ident[:],
                                                mb_sb[:, jj * 512:(jj + 1) * 512],
                                                start=False, stop=(extra == 0))
                                ats = {}
                                for e in (0, 1):
                                    at = atsb.tile([P, 1024], bf16,
                                                   name=f"at{e}", tag=f"at{e}")
                                    nc.scalar.activation(at[:, 0:w],
                                                         pss[e][:, 0:w],
                                                         AF.Exp, scale=SCALE)
                                    ats[e] = at
                                for idx, jj in enumerate(ch):
                                    vjj = 8 if jj == "T" else jj
                                    vcols = vext[:, vjj * (DH + 2):
                                                 (vjj + 1) * (DH + 2)]
                                    for e in (0, 1):
                                        nc.tensor.matmul(
                                            psos[e][0:DH + 2, :], vcols,
                                            ats[e][:, idx * 512:(idx + 1) * 512],
                                            start=first_av,
                                            stop=(jj == alljj[-1]))
                                    first_av = False
                            bsl0 = slice(b0 * 512, (b0 + 1) * 512)
                            for e in (1, 0):
                                aot = nrm.tile([DH + 2, 512], f32,
                                               name=f"aot{b0}{e}",
                                               tag=f"aot{b0}{e}")
                                nc.vector.tensor_copy(aot[:],
                                                      psos[e][0:DH + 2, :])
                                if e == 1:
                                    nc.vector.tensor_copy(rsb[DH:DH + 2, bsl0],
                                                          aot[DH:DH + 2, :])
                                else:
                                    nc.vector.tensor_copy(rsb[DH:DH + 1, bsl0],
                                                          aot[DH:DH + 1, :])
                                aots[(b0, e)] = aot
                        rows2 = rsb[DH:DH + 2, :]
                        nc.scalar.activation(rows2, rows2, AF.Ln)
                        nc.scalar.activation(rows2, rows2, AF.Exp, scale=-1.0)
                        for e in (0, 1):
                            bc = nrm.tile([P, N], f32, name=f"bc{e}",
                                          tag=f"bc{e}")
                            nc.vector.stream_shuffle(bc[DH:DH + ROT, :],
                                                     rsb[DH:DH + ROT, :],
                                                     [e] * 32)
                            nc.sync.dma_start(bc[0:ROT, :], bc[DH:DH + ROT, :])
                            nc.sync.dma_start(bc[ROT:DH, :], bc[0:ROT, :])
                            for b0 in range(IB):
                                osl = slice(pc * N + b0 * 512,
                                            pc * N + b0 * 512 + 512)
                                bsl = slice(b0 * 512, (b0 + 1) * 512)
                                src = aots[(b0, e)]
                                if e == 0:
                                    nc.gpsimd.tensor_tensor(
                                        out=ao[0:DH, osl], in0=src[0:DH, :],
                                        in1=bc[0:DH, bsl], op=OP.mult)
                                else:
                                    tmp = nrm.tile([DH, 512], bf16,
                                                   name="tmpn", tag="tmpn")
                                    nc.gpsimd.tensor_tensor(
                                        out=tmp[:], in0=src[0:DH, :],
                                        in1=bc[0:DH, bsl], op=OP.mult)
                                    nc.sync.dma_start(ao[DH:P, osl], tmp[:])

                # ---- Phase 4: out projection ----
                with tc.tile_pool(name="opps", bufs=4, space="PSUM") as opps, \
                     tc.tile_pool(name="opsb", bufs=3) as opsb:
                    for t in range(NT):
                        orow = opsb.tile([P, DIM], f32, tag="orow")
                        for nb2 in range(2):
                            ps = opps.tile([P, 512], f32, tag="op")
                            for p in range(PC):
                                nc.tensor.matmul(
                                    ps[:],
                                    ao[:, p * N + t * P: p * N + t * P + 128],
                                    wout_sb[:, p * DIM + nb2 * 512:
                                            p * DIM + nb2 * 512 + 512],
                                    start=(p == 0), stop=(p == PC - 1))
                            nc.scalar.copy(
                                orow[:, nb2 * 512:(nb2 + 1) * 512], ps[:])
                        nc.sync.dma_start(
                            d_out.ap()[b * N + t * P: b * N + (t + 1) * P, :],
                            orow[:])

    nc.compile()
    return nc


_PROG_CACHE = {}


def _get_program(mask_trivial, nb=NB):
    key = (nb, bool(mask_trivial))
    if key not in _PROG_CACHE:
        _PROG_CACHE[key] = _build_program(nb, key[1])
    return _PROG_CACHE[key]


def _host_prep(core, x, mask, freqs, ln_g, ln_b, W_q, W_kv, W_out, null_kv,
               mask_trivial, nb=NB):
    R = _blob_rows(nb, mask_trivial)
    blob = np.zeros((R["_total"], 1024), BF16)

    for i in range(nb):
        blob[R["x"] + i * N: R["x"] + (i + 1) * N, :] = x[core * nb + i]

    Wq_eff = W_q * ln_g[:, None]                        # [1024, 1024]
    Wkv_eff = W_kv * ln_g[:, None]                      # [1024, 128]
    bq = ln_b @ W_q                                     # [1024]
    bkv = ln_b @ W_kv                                   # [128]
    Wk, Wv = Wkv_eff[:, 0:DH], Wkv_eff[:, DH:2 * DH]
    bk, bv = bkv[0:DH], bkv[DH:2 * DH]

    blob[R["wq"]:R["wq"] + DIM, :] = Wq_eff
    blob[R["wkv"]:R["wkv"] + DIM, 0:DH] = Wk
    blob[R["wkv"]:R["wkv"] + DIM, DH:2 * DH] = Wk
    blob[R["wkv"]:R["wkv"] + DIM, 128:192] = Wv
    blob[R["wout"]:R["wout"] + DIM, :] = W_out

    f = np.asarray(freqs, np.float64)                   # [1024, 32]
    blob[R["cos"]:R["cos"] + P, :] = np.tile(np.cos(f).T, (4, 1))
    s = np.sin(f).T                                     # [32, 1024]
    sm = s.copy()
    sm[0:ROT // 2, :] = -s[0:ROT // 2, :]
    blob[R["sinm"]:R["sinm"] + P, :] = np.tile(sm, (4, 1))

    tri = np.zeros((P, 5 * 512), F32)
    pidx = np.arange(P)[:, None]
    il = np.arange(512)[None, :]
    for k in range(4):
        tri[:, k * 512:(k + 1) * 512] = np.where(il >= 128 * k + pidx,
                                                 0.0, NEG)
    tri[NN:, 4 * 512:5 * 512] = NEG
    blob[R["tri"]:R["tri"] + P, :] = tri[:, 0:1024]
    blob[R["tri"] + P:R["tri"] + 2 * P, :] = tri[:, 1024:2048]
    blob[R["tri"] + 2 * P:R["tri"] + 3 * P, 0:512] = tri[:, 2048:2560]

    nk = np.asarray(null_kv[0]).T                       # [64, 2]
    blob[R["misc"]:R["misc"] + DH, MC_KTAIL:MC_KTAIL + NN] = nk
    blob[R["misc"] + DH:R["misc"] + P, MC_KTAIL:MC_KTAIL + NN] = nk
    blob[R["misc"]:R["misc"] + P,
         MC_IDENT:MC_IDENT + P] = np.eye(P, dtype=F32)
    blob[R["misc"]:R["misc"] + NN, MC_VTAIL:MC_VTAIL + DH] = \
        np.asarray(null_kv[1])
    blob[R["misc"]:R["misc"] + NN, MC_VTAIL + DH:MC_VTAIL + DH + NN] = 1.0
    for p in range(PC):
        blob[R["misc"]:R["misc"] + P, MC_QB + p] = bq[p * 128:(p + 1) * 128]
    blob[R["misc"]:R["misc"] + P, MC_KB] = np.concatenate([bk, bk])
    blob[R["misc"]:R["misc"] + DH, MC_VB] = bv

    if not mask_trivial:
        for i in range(nb):
            mrow = np.where(np.asarray(mask[core * nb + i]), 0.0, NEG)
            mb = np.zeros((P, NT * 512), F32)
            for jj in range(NT):
                mb[:, jj * 512:(jj + 1) * 512] = \
                    mrow[jj * P:(jj + 1) * P][:, None]
            for band in range(4):
                blob[R["mb"] + (i * 4 + band) * P:
                     R["mb"] + (i * 4 + band + 1) * P, :] = \
                    mb[:, band * 1024:(band + 1) * 1024]

    return {"blob": blob}


def _run(x, mask, freqs, ln_g, ln_b, W_q, W_kv, W_out, null_kv, **spmd_kwargs):
    x = np.asarray(x, F32)
    mask = np.asarray(mask)
    freqs = np.asarray(freqs, F32)
    ln_g = np.asarray(ln_g, np.float64)
    ln_b = np.asarray(ln_b, np.float64)
    W_q = np.asarray(W_q, np.float64)
    W_kv = np.asarray(W_kv, np.float64)
    W_out = np.asarray(W_out, np.float64)
    null_kv = np.asarray(null_kv, F32)

    mask_trivial = bool(mask.all())
    nc = _get_program(mask_trivial)
    in_maps = [
        _host_prep(c, x, mask, freqs, ln_g, ln_b, W_q, W_kv, W_out, null_kv,
                   mask_trivial)
        for c in range(NCORES)
    ]
    res = bass_utils.run_bass_kernel_spmd(nc, in_maps, list(range(NCORES)),
                                          **spmd_kwargs)
    out = np.empty((B, N, DIM), F32)
    for b in range(B):
        out[b] = res.results[b // NB]["out"][(b % NB) * N:(b % NB + 1) * N]
    return out, res


def kernel(x, mask, freqs, ln_g, ln_b, W_q, W_kv, W_out, null_kv):
    out, _ = _run(x, mask, freqs, ln_g, ln_b, W_q, W_kv, W_out, null_kv)
    return out


# revision 53
# speedup vs baseline: 26.7361x; 1.1244x over previous
"""Trainium2 Bass kernel for nn_Attention (LN -> QKV proj -> partial RoPE ->
null-KV prepend -> causal MQA attention -> out proj).

Dispatch-cost-aware sharding: the axon PJRT path costs ~10ms fixed +
~0.9ms/core + ~0.8ms/buffer per exec, with payload bytes nearly free.
So: NCORES cores (default 2), each computing NB=4//NCORES full batches
(all 16 heads), with ONE packed f32 input blob + ONE f32 output tensor
per core. Output is a disjoint batch stack (no host reduction).

RoPE is applied post-projection via stream_shuffle partition rotation of
the biased q/k/v rows (rot contribution = shuffle * signed-sin + q * cos),
so no separate rot-weight projections are needed.

All compute ops keep uniform start-partitions (walrus checkSBSameStartPartition):
- k is projected twice (rows 0:64 and 64:128) so odd heads' QK matmuls run with
  lhsT/rhs both at base 64.
- rope groups live at rows base+(0:32) for base in {0, 64}; shuffles and
  combines stay within one base.
"""

import sys

for _p in ("/opt/trn_rl_repo",):
    if _p not in sys.path:
        sys.path.insert(0, _p)

import numpy as np
import ml_dtypes

import concourse.bass as bass
import concourse.tile as tile
from concourse import bacc, mybir
from concourse import bass_utils

F32 = np.float32
BF16 = ml_dtypes.bfloat16

B, N, DIM = 4, 1024, 1024
HEADS, DH = 16, 64
PC = HEADS // 2             # 8 head-pair groups, all on one core
ROT = 32
NN = 2                      # null kv
EPS = 1e-5
P = 128
NEG = -1.0e38
SCALE = DH ** -0.5
NT = N // P                 # 8 i-tiles / D-chunks
IB = N // 512               # 2 i-blocks

NB = 4                      # batches per core
NCORES = B // NB

dt = mybir.dt

ROT_SHUF = list(range(16, 32)) + list(range(0, 16))


def _chunks_for_block(b0):
    """j-tile chunks per i-block: lists of seq j-tile indices; 'T' = tail."""
    if b0 == 0:
        return [[0, 1], [2, 3], ["T"]]
    return [[0, 1], [2, 3], [4, 5], [6, 7], ["T"]]


def _prime_act_tables(arch):
    """Make Exp/Ln resolve to the single set containing both, so the
    act-table insertion pass emits one load instead of thrashing."""
    import concourse.hw_specs as hw_specs
    AF = mybir.ActivationFunctionType
    tables = hw_specs.get_activation_tables(arch)
    if "natural_log_exp_and_others" in tables:
        for name, fns in tables.items():
            if name != "natural_log_exp_and_others":
                for f in (AF.Exp, AF.Ln, AF.Square, AF.Identity, AF.Copy):
                    fns.discard(f)


def _blob_rows(nb, mask_trivial):
    """Row offsets of each section in the packed [R, 1024] f32 blob."""
    off = {}
    r = 0
    off["x"] = r; r += nb * N
    off["wq"] = r; r += DIM          # [1024, 1024]
    off["wkv"] = r; r += DIM         # cols 0:128 = [Wk|Wk], 128:192 = Wv
    off["wout"] = r; r += DIM        # [1024, 1024]
    off["cos"] = r; r += P           # [128, 1024]
    off["sinm"] = r; r += P          # signed sin, [128, 1024]
    off["tri"] = r; r += 3 * P       # [128,2560] as 3 bands (1024,1024,512)
    off["misc"] = r; r += P          # ktail|ident|vtail|qb|kb|vb
    if not mask_trivial:
        off["mb"] = r; r += nb * 4 * P   # per-batch [128, 4096] as 4 bands
    off["_total"] = r
    return off


# misc band column layout
MC_KTAIL = 0          # [128, 128]
MC_IDENT = 128        # [128, 128]
MC_VTAIL = 256        # [128, 66]
MC_QB = 322           # [128, 8] f32
MC_KB = 330           # [128, 1] f32
MC_VB = 331           # [64, 1] f32


def _build_program(nb, mask_trivial):
    nc = bacc.Bacc("TRN2", target_bir_lowering=False, debug=False)
    _prime_act_tables(nc.m.arch)

    f32, bf16 = dt.float32, dt.bfloat16
    AF = mybir.ActivationFunctionType
    OP = mybir.AluOpType

    R = _blob_rows(nb, mask_trivial)
    d_blob = nc.dram_tensor("blob", [R["_total"], 1024], bf16,
                            kind="ExternalInput")
    d_out = nc.dram_tensor("out", [nb * N, DIM], f32, kind="ExternalOutput")

    def bap(key, r0, r1, c0, c1):
        return d_blob.ap()[R[key] + r0: R[key] + r1, c0:c1]

    with tile.TileContext(nc) as tc:
        from contextlib import ExitStack

        ctx = ExitStack()
        with ctx:
            consts = ctx.enter_context(tc.tile_pool(name="consts", bufs=1))
            persist = ctx.enter_context(tc.tile_pool(name="persist", bufs=1))

            # ---- persistent SBUF tensors ----
            wq_sb = consts.tile([P, NT * 1024], bf16)      # 8 chunks x [128,1024]
            wkk_sb = consts.tile([P, NT * 128], bf16)
            wv_sb = consts.tile([P, NT * 64], bf16)
            wout_sb = consts.tile([P, PC * DIM], bf16)     # 8 pair chunks
            cos_sb = consts.tile([P, N], bf16)
            sinm_sb = consts.tile([P, N], bf16)
            tri_sb = consts.tile([P, 5 * 512], bf16)
            ktail_sb = consts.tile([P, P], bf16)
            vtail_sb = consts.tile([P, DH + 2], bf16)
            ident = consts.tile([P, P], bf16)
            qb_sb = consts.tile([P, PC], f32)
            kb_sb = consts.tile([P, 1], f32)
            vb_sb = consts.tile([DH, 1], f32)
            mb_sb = None
            if not mask_trivial:
                mb_sb = persist.tile([P, NT * 512], bf16)

            qp = persist.tile([P, PC * N], bf16)           # q pairs [128, i]
            kT = persist.tile([P, N], bf16)                # k duplicated rows
            vT = persist.tile([DH, N], bf16)
            vext = persist.tile([P, 9 * (DH + 2)], bf16)   # v + dual ones cols
            ao = persist.tile([P, PC * N], bf16)           # attn out pairs

            # ---- load weights (bf16 blob -> SBUF, direct DMA) ----
            with tc.tile_pool(name="wstg", bufs=2) as stg:
                for c in range(NT):
                    nc.sync.dma_start(wq_sb[:, c * 1024:(c + 1) * 1024],
                                      bap("wq", c * P, (c + 1) * P, 0, 1024))
                    nc.sync.dma_start(wkk_sb[:, c * 128:(c + 1) * 128],
                                      bap("wkv", c * P, (c + 1) * P, 0, 128))
                    nc.sync.dma_start(wv_sb[:, c * 64:(c + 1) * 64],
                                      bap("wkv", c * P, (c + 1) * P, 128, 192))
                for p in range(PC):
                    nc.sync.dma_start(wout_sb[:, p * DIM:(p + 1) * DIM],
                                      bap("wout", p * P, (p + 1) * P, 0, 1024))
                nc.sync.dma_start(cos_sb[:], bap("cos", 0, P, 0, 1024))
                nc.sync.dma_start(sinm_sb[:], bap("sinm", 0, P, 0, 1024))
                nc.sync.dma_start(tri_sb[:, 0:1024], bap("tri", 0, P, 0, 1024))
                nc.sync.dma_start(tri_sb[:, 1024:2048],
                                  bap("tri", P, 2 * P, 0, 1024))
                nc.sync.dma_start(tri_sb[:, 2048:2560],
                                  bap("tri", 2 * P, 3 * P, 0, 512))
                nc.sync.dma_start(ktail_sb[:],
                                  bap("misc", 0, P, MC_KTAIL, MC_KTAIL + P))
                nc.sync.dma_start(ident[:],
                                  bap("misc", 0, P, MC_IDENT, MC_IDENT + P))
                nc.sync.dma_start(vtail_sb[:],
                                  bap("misc", 0, P, MC_VTAIL, MC_VTAIL + DH + 2))
                bst = stg.tile([P, 16], bf16, tag="bst", name="bst")
                nc.sync.dma_start(bst[:, 0:PC],
                                  bap("misc", 0, P, MC_QB, MC_QB + PC))
                nc.sync.dma_start(bst[:, PC:PC + 1],
                                  bap("misc", 0, P, MC_KB, MC_KB + 1))
                nc.sync.dma_start(bst[0:DH, PC + 1:PC + 2],
                                  bap("misc", 0, DH, MC_VB, MC_VB + 1))
                nc.vector.tensor_copy(qb_sb[:], bst[:, 0:PC])
                nc.vector.tensor_copy(kb_sb[:], bst[:, PC:PC + 1])
                nc.vector.tensor_copy(vb_sb[:], bst[0:DH, PC + 1:PC + 2])

            # ---- helpers (same structure as 8-head version, PC=8) ----
            def ln_reduce_tile(ph1, t, xt, rsums, accs):
                c4 = t % 4
                nc.vector.tensor_reduce(rsums[:, c4:c4 + 1], xt[:],
                                        axis=mybir.AxisListType.X, op=OP.add)
                sq = ph1.tile([P, DIM], bf16, tag="sq", name="sq")
                nc.scalar.activation(sq[:], xt[:], AF.Square,
                                     accum_out=accs[:, c4:c4 + 1])

            def ln_stats_batch(stp, rsums, accs):
                mean = stp.tile([P, 4], f32, tag="stb", name="mean")
                nc.vector.tensor_scalar(out=mean[:], in0=rsums[:],
                                        scalar1=1.0 / DIM, scalar2=None,
                                        op0=OP.mult)
                ex2 = stp.tile([P, 4], f32, tag="stb", name="ex2")
                nc.vector.tensor_scalar(out=ex2[:], in0=accs[:],
                                        scalar1=1.0 / DIM, scalar2=None,
                                        op0=OP.mult)
                var = stp.tile([P, 4], f32, tag="stb", name="var")
                nc.vector.scalar_tensor_tensor(
                    out=var[:], in0=mean[:], scalar=-1.0, in1=mean[:],
                    op0=OP.mult, op1=OP.mult)
                nc.vector.scalar_tensor_tensor(
                    out=var[:], in0=ex2[:], scalar=EPS, in1=var[:],
                    op0=OP.add, op1=OP.add)
                nc.scalar.activation(var[:], var[:], AF.Ln)
                rstd = stp.tile([P, 4], f32, tag="stb", name="rstd")
                nc.scalar.activation(rstd[:], var[:], AF.Exp, scale=-0.5)
                negmr = stp.tile([P, 4], f32, tag="stb", name="negmr")
                nc.vector.scalar_tensor_tensor(
                    out=negmr[:], in0=mean[:], scalar=-1.0, in1=rstd[:],
                    op0=OP.mult, op1=OP.mult)
                return rstd, negmr

            def ln_xn_tile(xnT, ph1, ps1, t, xt, rstd, negmr):
                c4 = t % 4
                xn = ph1.tile([P, DIM], bf16, tag="xn", name="xn")
                nc.vector.tensor_scalar(out=xn[:], in0=xt[:],
                                        scalar1=rstd[:, c4:c4 + 1],
                                        scalar2=negmr[:, c4:c4 + 1],
                                        op0=OP.mult, op1=OP.add)
                for g in range(2):
                    pst = ps1.tile([P, 512], bf16, tag="tp", name="pst")
                    for c4b in range(4):
                        c = g * 4 + c4b
                        nc.tensor.transpose(pst[:, c4b * P:(c4b + 1) * P],
                                            xn[:, c * P:(c + 1) * P], ident[:])
                    dest = xnT[:].rearrange("p (c i) -> p c i", c=NT)[
                        :, g * 4:(g + 1) * 4, t * P:(t + 1) * P]
                    src = pst[:].rearrange("p (c i) -> p c i", c=4)
                    nc.scalar.copy(dest, src)

            def mm_proj(xnT, ps2, w_sb, wwidth, col0, cols, ib, rows=P):
                ps = ps2.tile([P, 512], f32, tag="proj", name="ps")
                for c in range(NT):
                    nc.tensor.matmul(
                        ps[0:rows, :],
                        w_sb[:, c * wwidth + col0: c * wwidth + col0 + cols],
                        xnT[:, c * N + ib * 512: c * N + ib * 512 + 512],
                        start=(c == 0), stop=(c == NT - 1))
                return ps

            def rope_rows(rp, dst, base, isl_c, sin_cols):
                """dst rows base:base+32 (cols isl_c slice of width 512):
                dst = dst*cos + shuffle(dst)*sinm."""
                rsl = slice(base, base + ROT)
                tmp = rp.tile([P, 512], bf16, tag="rt", name="rt")
                nc.vector.stream_shuffle(tmp[rsl, :], dst[rsl, isl_c], ROT_SHUF)
                nc.vector.tensor_tensor(out=dst[rsl, isl_c],
                                        in0=dst[rsl, isl_c],
                                        in1=cos_sb[rsl, sin_cols], op=OP.mult)
                nc.vector.tensor_tensor(out=tmp[rsl, :], in0=tmp[rsl, :],
                                        in1=sinm_sb[rsl, sin_cols], op=OP.mult)
                nc.vector.tensor_tensor(out=dst[rsl, isl_c],
                                        in0=dst[rsl, isl_c],
                                        in1=tmp[rsl, :], op=OP.add)

            def proj_ib(xnT, ps2, vtp, rp, ib):
                isl = slice(ib * 512, (ib + 1) * 512)
                for p in range(PC):
                    csl = slice(p * N + ib * 512, p * N + ib * 512 + 512)
                    ps = mm_proj(xnT, ps2, wq_sb, 1024, p * P, P, ib)
                    nc.scalar.add(qp[:, csl], ps[:], qb_sb[:, p:p + 1])
                    for base in (0, DH):
                        rope_rows(rp, qp, base, csl, isl)
                ps = mm_proj(xnT, ps2, wkk_sb, 128, 0, P, ib)
                nc.scalar.add(kT[:, isl], ps[:], kb_sb[:])
                for base in (0, DH):
                    rope_rows(rp, kT, base, isl, isl)
                ps = mm_proj(xnT, ps2, wv_sb, 64, 0, DH, ib, rows=DH)
                nc.scalar.add(vT[:, isl], ps[0:DH, :], vb_sb[:])
                rope_rows(rp, vT, 0, isl, isl)
                # v row-major + dual ones cols for this i-block's j-tiles
                for jj in range(ib * 4, ib * 4 + 4):
                    pv = vtp.tile([P, DH], bf16, tag="vt", name="pv")
                    nc.tensor.transpose(pv[:], vT[:, jj * P:(jj + 1) * P],
                                        ident[0:DH, 0:DH])
                    vbase = jj * (DH + 2)
                    nc.vector.tensor_copy(vext[:, vbase:vbase + DH], pv[:])
                    nc.vector.memset(vext[:, vbase + DH:vbase + DH + 2], 1.0)

            # ================= per-batch pipeline =================
            for b in range(nb):
                if not mask_trivial:
                    for band in range(4):
                        nc.sync.dma_start(
                            mb_sb[:, band * 1024:(band + 1) * 1024],
                            bap("mb", (b * 4 + band) * P,
                                (b * 4 + band + 1) * P, 0, 1024))

                # ---- Phases 1+2: LN + projections + rope ----
                with tc.tile_pool(name="ph1sb", bufs=4) as ph1, \
                     tc.tile_pool(name="ph1st", bufs=32) as stp, \
                     tc.tile_pool(name="xnp", bufs=1) as xnp, \
                     tc.tile_pool(name="ph1ps", bufs=2, space="PSUM") as ps1, \
                     tc.tile_pool(name="ph2ps", bufs=5, space="PSUM") as ps2, \
                     tc.tile_pool(name="rope", bufs=4) as rp, \
                     tc.tile_pool(name="vtp", bufs=1, space="PSUM") as vtp:
                    xnT = xnp.tile([P, NT * N], bf16, tag="xnT", name="xnT")
                    xts = []
                    for t in range(NT):
                        xt = ph1.tile([P, DIM], bf16, tag=f"x{t % 4}",
                                      name=f"xt{t}", bufs=2)
                        nc.gpsimd.dma_start(
                            xt[:], bap("x", b * N + t * P, b * N + (t + 1) * P,
                                       0, 1024))
                        xts.append(xt)
                    for half in range(2):
                        rsums = stp.tile([P, 4], f32, tag=f"rs{half}",
                                         name=f"rsums{half}", bufs=1)
                        accs = stp.tile([P, 4], f32, tag=f"ac{half}",
                                        name=f"accs{half}", bufs=1)
                        for t in range(half * 4, half * 4 + 4):
                            ln_reduce_tile(ph1, t, xts[t], rsums, accs)
                        rstd, negmr = ln_stats_batch(stp, rsums, accs)
                        for t in range(half * 4, half * 4 + 4):
                            ln_xn_tile(xnT, ph1, ps1, t, xts[t], rstd, negmr)
                        proj_ib(xnT, ps2, vtp, rp, half)
                    nc.vector.tensor_copy(vext[:, 8 * (DH + 2):9 * (DH + 2)],
                                          vtail_sb[:])

                # ---- Phase 3: attention (pair-packed) ----
                with tc.tile_pool(name="simps", bufs=3, space="PSUM") as simps, \
                     tc.tile_pool(name="outps", bufs=1, space="PSUM") as outps, \
                     tc.tile_pool(name="atsb", bufs=6) as atsb, \
                     tc.tile_pool(name="nrm", bufs=3) as nrm:
                    for pc in range(PC):
                        rsb = nrm.tile([P, N], f32, name="rsb", tag="rsb")
                        nc.vector.memset(rsb[DH:DH + ROT, :], 1.0)
                        aots = {}
                        for b0 in range(IB):
                            chunks = _chunks_for_block(b0)
                            alljj = [jj for ch in chunks for jj in ch]
                            qhs = {}
                            psos = {}
                            for e in (0, 1):
                                hb = e * DH
                                qhs[e] = qp[hb:hb + DH,
                                            pc * N + b0 * 512:
                                            pc * N + b0 * 512 + 512]
                                psos[e] = outps.tile([P, 512], f32,
                                                     name=f"pso{e}",
                                                     tag=f"outT{e}")
                            first_av = True
                            for ch in chunks:
                                w = len(ch) * 512
                                pss = {}
                                for e in (0, 1):
                                    pss[e] = simps.tile([P, 1024], f32,
                                                        name=f"pss{e}",
                                                        tag="sim")
                                for idx, jj in enumerate(ch):
                                    for e in (0, 1):
                                        hb = e * DH
                                        seg = pss[e][:, idx * 512:(idx + 1) * 512]
                                        diag = jj != "T" and jj >= 4 * b0
                                        extra = (1 if jj == "T" else
                                                 (1 if diag else 0)
                                                 + (0 if mask_trivial else 1))
                                        if jj == "T":
                                            nc.tensor.matmul(
                                                seg, ktail_sb[hb:hb + DH, :],
                                                qhs[e], start=True, stop=False)
                                        else:
                                            nc.tensor.matmul(
                                                seg,
                                                kT[hb:hb + DH,
                                                   jj * P:(jj + 1) * P],
                                                qhs[e], start=True,
                                                stop=(extra == 0))
                                for idx, jj in enumerate(ch):
                                    for e in (0, 1):
                                        seg = pss[e][:, idx * 512:(idx + 1) * 512]
                                        if jj == "T":
                                            nc.tensor.matmul(
                                                seg, ident[:],
                                                tri_sb[:, 4 * 512:5 * 512],
                                                start=False, stop=True)
                                            continue
                                        diag = jj >= 4 * b0
                                        extra = ((1 if diag else 0)
                                                 + (0 if mask_trivial else 1))
                                        if diag:
                                            k = jj - 4 * b0
                                            extra -= 1
                                            nc.tensor.matmul(
                                                seg, ident[:],
                                                tri_sb[:, k * 512:(k + 1) * 512],
                                                start=False, stop=(extra == 0))
                                        if not mask_trivial:
                                            extra -= 1
                                            nc.tensor.matmul(
                                                seg, ident[:],
                                                mb_sb[:, jj * 512:(jj + 1) * 512],
                                                start=False, stop=(extra == 0))
                                ats = {}
                                for e in (0, 1):
                                    at = atsb.tile([P, 1024], bf16,
                                                   name=f"at{e}", tag=f"at{e}")
                                    nc.scalar.activation(at[:, 0:w],
                                                         pss[e][:, 0:w],
                                                         AF.Exp, scale=SCALE)
                                    ats[e] = at
                                for idx, jj in enumerate(ch):
                                    vjj = 8 if jj == "T" else jj
                                    vcols = vext[:, vjj * (DH + 2):
                                                 (vjj + 1) * (DH + 2)]
                                    for e in (0, 1):
                                        nc.tensor.matmul(
                                            psos[e][0:DH + 2, :], vcols,
                                            ats[e][:, idx * 512:(idx + 1) * 512],
                                            start=first_av,
                                            stop=(jj == alljj[-1]))
                                    first_av = False
                            bsl0 = slice(b0 * 512, (b0 + 1) * 512)
                            for e in (1, 0):
                                aot = nrm.tile([DH + 2, 512], f32,
                                               name=f"aot{b0}{e}",
                                               tag=f"aot{b0}{e}")
                                nc.vector.tensor_copy(aot[:],
                                                      psos[e][0:DH + 2, :])
                                if e == 1:
                                    nc.vector.tensor_copy(rsb[DH:DH + 2, bsl0],
                                                          aot[DH:DH + 2, :])
                                else:
                                    nc.vector.tensor_copy(rsb[DH:DH + 1, bsl0],
                                                          aot[DH:DH + 1, :])
                                aots[(b0, e)] = aot
                        rows2 = rsb[DH:DH + 2, :]
                        nc.scalar.activation(rows2, rows2, AF.Ln)
                        nc.scalar.activation(rows2, rows2, AF.Exp, scale=-1.0)
                        for e in (0, 1):
                            bc = nrm.tile([P, N], f32, name=f"bc{e}",
                                          tag=f"bc{e}")
                            nc.vector.stream_shuffle(bc[DH:DH + ROT, :],
                                                     rsb[DH:DH + ROT, :],
                                                     [e] * 32)
                            nc.sync.dma_start(bc[0:ROT, :], bc[DH:DH + ROT, :])
                            nc.sync.dma_start(bc[ROT:DH, :], bc[0:ROT, :])
                            for b0 in range(IB):
                                osl = slice(pc * N + b0 * 512,
                                            pc * N + b0 * 512 + 512)
                                bsl = slice(b0 * 512, (b0 + 1) * 512)
                                src = aots[(b0, e)]
                                if e == 0:
                                    nc.gpsimd.tensor_tensor(
                                        out=ao[0:DH, osl], in0=src[0:DH, :],
                                        in1=bc[0:DH, bsl], op=OP.mult)
                                else:
                                    tmp = nrm.tile([DH, 512], bf16,
                                                   name="tmpn", tag="tmpn")
                                    nc.gpsimd.tensor_tensor(
                                        out=tmp[:], in0=src[0:DH, :],
                                        in1=bc[0:DH, bsl], op=OP.mult)
                                    nc.sync.dma_start(ao[DH:P, osl], tmp[:])

                # ---- Phase 4: out projection ----
                with tc.tile_pool(name="opps", bufs=4, space="PSUM") as opps, \
                     tc.tile_pool(name="opsb", bufs=3) as opsb:
                    for t in range(NT):
                        orow = opsb.tile([P, DIM], f32, tag="orow")
                        for nb2 in range(2):
                            ps = opps.tile([P, 512], f32, tag="op")
                            for p in range(PC):
                                nc.tensor.matmul(
                                    ps[:],
                                    ao[:, p * N + t * P: p * N + t * P + 128],
                                    wout_sb[:, p * DIM + nb2 * 512:
                                            p * DIM + nb2 * 512 + 512],
                                    start=(p == 0), stop=(p == PC - 1))
                            nc.scalar.copy(
                                orow[:, nb2 * 512:(nb2 + 1) * 512], ps[:])
                        nc.sync.dma_start(
                            d_out.ap()[b * N + t * P: b * N + (t + 1) * P, :],
                            orow[:])

    nc.compile()
    return nc


_PROG_CACHE = {}


def _get_program(mask_trivial, nb=NB):
    key = (nb, bool(mask_trivial))
    if key not in _PROG_CACHE:
        _PROG_CACHE[key] = _build_program(nb, key[1])
    return _PROG_CACHE[key]


def _host_prep(core, x, mask, freqs, ln_g, ln_b, W_q, W_kv, W_out, null_kv,
               mask_trivial, nb=NB):
    R = _blob_rows(nb, mask_trivial)
    blob = np.zeros((R["_total"], 1024), BF16)

    for i in range(nb):
        blob[R["x"] + i * N: R["x"] + (i + 1) * N, :] = x[core * nb + i]

    Wq_eff = W_q * ln_g[:, None]                        # [1024, 1024]
    Wkv_eff = W_kv * ln_g[:, None]                      # [1024, 128]
    bq = ln_b @ W_q                                     # [1024]
    bkv = ln_b @ W_kv                                   # [128]
    Wk, Wv = Wkv_eff[:, 0:DH], Wkv_eff[:, DH:2 * DH]
    bk, bv = bkv[0:DH], bkv[DH:2 * DH]

    blob[R["wq"]:R["wq"] + DIM, :] = Wq_eff
    blob[R["wkv"]:R["wkv"] + DIM, 0:DH] = Wk
    blob[R["wkv"]:R["wkv"] + DIM, DH:2 * DH] = Wk
    blob[R["wkv"]:R["wkv"] + DIM, 128:192] = Wv
    blob[R["wout"]:R["wout"] + DIM, :] = W_out

    f = np.asarray(freqs, np.float64)                   # [1024, 32]
    blob[R["cos"]:R["cos"] + P, :] = np.tile(np.cos(f).T, (4, 1))
    s = np.sin(f).T                                     # [32, 1024]
    sm = s.copy()
    sm[0:ROT // 2, :] = -s[0:ROT // 2, :]
    blob[R["sinm"]:R["sinm"] + P, :] = np.tile(sm, (4, 1))

    tri = np.zeros((P, 5 * 512), F32)
    pidx = np.arange(P)[:, None]
    il = np.arange(512)[None, :]
    for k in range(4):
        tri[:, k * 512:(k + 1) * 512] = np.where(il >= 128 * k + pidx,
                                                 0.0, NEG)
    tri[NN:, 4 * 512:5 * 512] = NEG
    blob[R["tri"]:R["tri"] + P, :] = tri[:, 0:1024]
    blob[R["tri"] + P:R["tri"] + 2 * P, :] = tri[:, 1024:2048]
    blob[R["tri"] + 2 * P:R["tri"] + 3 * P, 0:512] = tri[:, 2048:2560]

    nk = np.asarray(null_kv[0]).T                       # [64, 2]
    blob[R["misc"]:R["misc"] + DH, MC_KTAIL:MC_KTAIL + NN] = nk
    blob[R["misc"] + DH:R["misc"] + P, MC_KTAIL:MC_KTAIL + NN] = nk
    blob[R["misc"]:R["misc"] + P,
         MC_IDENT:MC_IDENT + P] = np.eye(P, dtype=F32)
    blob[R["misc"]:R["misc"] + NN, MC_VTAIL:MC_VTAIL + DH] = \
        np.asarray(null_kv[1])
    blob[R["misc"]:R["misc"] + NN, MC_VTAIL + DH:MC_VTAIL + DH + NN] = 1.0
    for p in range(PC):
        blob[R["misc"]:R["misc"] + P, MC_QB + p] = bq[p * 128:(p + 1) * 128]
    blob[R["misc"]:R["misc"] + P, MC_KB] = np.concatenate([bk, bk])
    blob[R["misc"]:R["misc"] + DH, MC_VB] = bv

    if not mask_trivial:
        for i in range(nb):
            mrow = np.where(np.asarray(mask[core * nb + i]), 0.0, NEG)
            mb = np.zeros((P, NT * 512), F32)
            for jj in range(NT):
                mb[:, jj * 512:(jj + 1) * 512] = \
                    mrow[jj * P:(jj + 1) * P][:, None]
            for band in range(4):
                blob[R["mb"] + (i * 4 + band) * P:
                     R["mb"] + (i * 4 + band + 1) * P, :] = \
                    mb[:, band * 1024:(band + 1) * 1024]

    return {"blob": blob}


def _run(x, mask, freqs, ln_g, ln_b, W_q, W_kv, W_out, null_kv, **spmd_kwargs):
    x = np.asarray(x, F32)
    mask = np.asarray(mask)
    freqs = np.asarray(freqs, F32)
    ln_g = np.asarray(ln_g, np.float64)
    ln_b = np.asarray(ln_b, np.float64)
    W_q = np.asarray(W_q, np.float64)
    W_kv = np.asarray(W_kv, np.float64)
    W_out = np.asarray(W_out, np.float64)
    null_kv = np.asarray(null_kv, F32)

    mask_trivial = bool(mask.all())
    nc = _get_program(mask_trivial)
    in_maps = [
        _host_prep(c, x, mask, freqs, ln_g, ln_b, W_q, W_kv, W_out, null_kv,
                   mask_trivial)
        for c in range(NCORES)
    ]
    res = bass_utils.run_bass_kernel_spmd(nc, in_maps, list(range(NCORES)),
                                          **spmd_kwargs)
    out = np.empty((B, N, DIM), F32)
    for b in range(B):
        out[b] = res.results[b // NB]["out"][(b % NB) * N:(b % NB + 1) * N]
    return out, res


def kernel(x, mask, freqs, ln_g, ln_b, W_q, W_kv, W_out, null_kv):
    out, _ = _run(x, mask, freqs, ln_g, ln_b, W_q, W_kv, W_out, null_kv)
    return out
